# revision 9
# baseline (speedup 1.0000x reference)
"""CPPN dense-MLP Trainium2 kernel.

Network (per point): 3 -> 16 (tanh) -> 8 x [16 -> 16 (tanh)] -> 1 (sigmoid).
2,097,152 points, pure data parallel across 8 NeuronCores.

Per-core layout: the core's 262,144 points are split into S=8 streams of
32,768 points.  Activations live in SBUF/PSUM "block layout": partition
16*j + m holds feature m of stream j, free dim indexes points within the
stream.  Every layer is then a single 128x128 block-diagonal stationary
matmul on the tensor engine (8 independent 16x16 matmuls per cycle).

Layer 0 (K=3) needs x with features on partitions; the host pre-transposes
each core's shard to [24, 32768] (partition 8f+j = feature f of stream j;
a cheap numpy reshape, part of sharding), so layer 0 is a single K=24
matmul with fully contiguous DMA loads.  (Alternative modes kept for A/B:
"three_mm" = stride-3 rhs APs over natural-layout x, ~+110us PE;
"strided" = DMA-side transpose, ~+600us of 4-byte-element descriptors.)

Activations (tanh / sigmoid + bias) run on the scalar engine directly from
PSUM into SBUF; tanh and sigmoid share one ACT table set so there are no
table reloads.

Matmuls are full float32 (4 cycles/column): this network doubles any
injected error every layer (~x250 over the 9-layer chain), so float32r's
~2^-13 per-product noise lands at ~0.3 absolute output error while fp32
stays at ~3e-5.  fp32 is mandatory for the gate, making the kernel
PE-bound.
"""

import numpy as np
import ml_dtypes
from contextlib import ExitStack


def round_f32r(a):
    """Round fp32 to the float32r grid (value representable as bf16 + bf16)."""
    a = np.asarray(a, np.float32)
    hi = a.astype(ml_dtypes.bfloat16).astype(np.float32)
    lo = (a - hi).astype(ml_dtypes.bfloat16).astype(np.float32)
    return hi + lo

import concourse.bass as bass
import concourse.tile as tile
from concourse import bacc, mybir
from concourse.bass_utils import run_bass_kernel_spmd

F32 = mybir.dt.float32
F32R = mybir.dt.float32r

N_FULL = 2097152
N_CORES = 8
N_CORE = N_FULL // N_CORES  # 262144 points per core
S = 8                       # streams per core
W = 16                      # hidden width
N_HIDDEN = 8


def format_inputs(W0, b0, Wh, bh, Wo, bo):
    """Build the block-diagonal stationary matrices + bias table (numpy)."""
    W0 = np.asarray(W0, np.float32)
    b0 = np.asarray(b0, np.float32)
    Wh = np.asarray(Wh, np.float32)
    bh = np.asarray(bh, np.float32)
    Wo = np.asarray(Wo, np.float32)
    bo = np.asarray(bo, np.float32)

    # Layer 0: three [S, 128] stationaries (one per input feature), packed
    # side by side into [S, 3*128].  stationary_f[j, 16j+m] = W0[m, f].
    w0f = np.zeros((S, 3 * 128), np.float32)
    for f in range(3):
        for j in range(S):
            w0f[j, f * 128 + 16 * j:f * 128 + 16 * j + W] = W0[:, f]

    # Hidden layers: [128, 8*128]; slice l is blockdiag(Wh[l].T x8):
    # stat[16j+fi, 16j+m] = Wh[l][m, fi].
    wh = np.zeros((128, N_HIDDEN * 128), np.float32)
    for l in range(N_HIDDEN):
        for j in range(S):
            r = 16 * j
            wh[r:r + W, l * 128 + r:l * 128 + r + W] = Wh[l].T

    # Output layer: [128, S]: stat[16j+fi, j] = Wo[0, fi].
    wo = np.zeros((128, S), np.float32)
    for j in range(S):
        wo[16 * j:16 * j + W, j] = Wo[0, :]

    # Bias table [128, 10]: col 0 = b0 block, cols 1..8 = bh blocks,
    # col 9 rows 0..7 = bo.
    bias = np.zeros((128, 10), np.float32)
    for j in range(S):
        bias[16 * j:16 * j + W, 0] = b0
        for l in range(N_HIDDEN):
            bias[16 * j:16 * j + W, 1 + l] = bh[l]
    bias[0:S, 9] = bo[0]

    # Layer 0 alternative: single [24, 128] stationary for K=24 matmul
    # over a DMA-transposed x (f-major partitions): w0t[8f+j, 16j+m] = W0[m, f].
    w0t = np.zeros((24, 128), np.float32)
    for j in range(S):
        for f in range(3):
            w0t[8 * f + j, 16 * j:16 * j + W] = W0[:, f]

    return {"w0f": w0f, "w0t": w0t, "wh": wh, "wo": wo, "bias": bias}


def build_program(n_core=N_CORE, g_cols=1024, f_cols=512, repeat=1,
                  num_devices=N_CORES, depth=2, xbufs=None, hbufs=None,
                  ybufs=None, zbufs=None, l0_mode="three_mm"):
    """Build + compile the per-core Bass program (SPMD: same on all cores).

    n_core: points per core.  g_cols: free-dim columns per group (pipeline
    granularity; one ACT instruction per layer per group).  f_cols: free-dim
    columns per matmul (<= 512, one PSUM bank).  repeat: run the whole
    kernel body this many times (for wall-clock timing; output idempotent).
    """
    stream_len = n_core // S
    ng = stream_len // g_cols
    assert stream_len % g_cols == 0 and g_cols % f_cols == 0
    cpg = g_cols // f_cols  # matmul chunks per group

    nc = bacc.Bacc("TRN2", target_bir_lowering=False, debug=False,
                   num_devices=num_devices)
    if l0_mode == "host_t":
        xt_ap = nc.dram_tensor("xt", [24, n_core // S], F32,
                               kind="ExternalInput").ap()
        x_ap = None
    else:
        x_ap = nc.dram_tensor("x", [n_core, 3], F32,
                              kind="ExternalInput").ap()
    w0f_ap = nc.dram_tensor("w0f", [S, 3 * 128], F32, kind="ExternalInput").ap()
    w0t_ap = nc.dram_tensor("w0t", [24, 128], F32, kind="ExternalInput").ap()
    wh_ap = nc.dram_tensor("wh", [128, N_HIDDEN * 128], F32,
                           kind="ExternalInput").ap()
    wo_ap = nc.dram_tensor("wo", [128, S], F32, kind="ExternalInput").ap()
    bias_ap = nc.dram_tensor("bias", [128, 10], F32, kind="ExternalInput").ap()
    y_ap = nc.dram_tensor("y", [n_core, 1], F32, kind="ExternalOutput").ap()

    # DRAM views: stream-major.  x[(j n) f] -> [S, stream_len*3] so each
    # partition's group slice is one contiguous run.
    if x_ap is not None:
        xr = x_ap.rearrange("(j n) f -> j (n f)", j=S)
        # transposed view for strided-DMA layer 0: [S, 3, stream_len]
        xt_dram = x_ap.rearrange("(j n) f -> j f n", j=S)
    yr = y_ap.rearrange("(j n) o -> j (n o)", j=S)

    Tanh = mybir.ActivationFunctionType.Tanh
    Sigmoid = mybir.ActivationFunctionType.Sigmoid

    with tile.TileContext(nc) as tc, ExitStack() as ctx:
        if zbufs is None:
            # slots are bank (2KB) granular; fill the 8 banks
            banks_per_slot = max(1, (g_cols * 4) // 2048)
            zbufs = max(2, 8 // banks_per_slot)
        if xbufs is None:
            xbufs = depth + 1
        if hbufs is None:
            hbufs = 2 * depth + 1
        if ybufs is None:
            ybufs = depth + 1
        consts = ctx.enter_context(tc.tile_pool(name="consts", bufs=1))
        xpool = ctx.enter_context(tc.tile_pool(name="xpool", bufs=xbufs))
        hpool = ctx.enter_context(tc.tile_pool(name="hpool", bufs=hbufs))
        ypool = ctx.enter_context(tc.tile_pool(name="ypool", bufs=ybufs))
        zpool = ctx.enter_context(
            tc.tile_pool(name="zpool", bufs=zbufs, space="PSUM"))

        w0f_sb = consts.tile([S, 3 * 128], F32)
        nc.sync.dma_start(w0f_sb[:], w0f_ap[:])
        w0t_sb = consts.tile([24, 128], F32)
        nc.sync.dma_start(w0t_sb[:], w0t_ap[:])
        wh_sb = consts.tile([128, N_HIDDEN * 128], F32)
        nc.sync.dma_start(wh_sb[:], wh_ap[:])
        wo_sb = consts.tile([128, S], F32)
        nc.sync.dma_start(wo_sb[:], wo_ap[:])
        bias_sb = consts.tile([128, 10], F32)
        nc.sync.dma_start(bias_sb[:], bias_ap[:])

        # Interleave `depth` groups at each layer step: within a group the
        # PE's layer l+1 strictly follows ACT of layer l, so a single group
        # serializes PE<->ACT.  Emitting layer l for D groups back-to-back
        # gives the PE work while ACT drains the other groups' PSUM tiles.
        for _rep in range(repeat):
            for g0 in range(0, ng, depth):
                gs = range(g0, min(g0 + depth, ng))
                xss, hs, zs = {}, {}, {}
                for g in gs:
                    if l0_mode == "three_mm":
                        # ---- load x chunk [S, 3*g_cols], contiguous/stream
                        xs = xpool.tile([S, 3 * g_cols], F32, tag="xs")
                        nc.sync.dma_start(
                            xs[:], xr[:, g * 3 * g_cols:(g + 1) * 3 * g_cols])
                        # stride-3 feature view: [S, g_cols, 3]
                        xss[g] = xs.rearrange("p (n f) -> p n f", f=3)
                    elif l0_mode == "host_t":
                        # ---- x pre-transposed on host: contiguous [24, g]
                        xs = xpool.tile([24, g_cols], F32, tag="xs")
                        nc.sync.dma_start(
                            xs[:], xt_ap[:, g * g_cols:(g + 1) * g_cols])
                        xss[g] = xs
                    else:
                        # ---- strided-DMA transpose load: [24, g_cols]
                        xs = xpool.tile([24, g_cols], F32, tag="xs")
                        for f in range(3):
                            nc.sync.dma_start(
                                xs[8 * f:8 * (f + 1), :],
                                xt_dram[:, f, g * g_cols:(g + 1) * g_cols])
                        xss[g] = xs

                # ---- layer 0
                for g in gs:
                    z = zpool.tile([128, g_cols], F32, tag="z")
                    zs[g] = z
                    for c in range(cpg):
                        if l0_mode == "three_mm":
                            for f in range(3):
                                nc.tensor.matmul(
                                    z[:, c * f_cols:(c + 1) * f_cols],
                                    lhsT=w0f_sb[:, f * 128:(f + 1) * 128],
                                    rhs=xss[g][:, c * f_cols:(c + 1) * f_cols,
                                               f],
                                    start=(f == 0), stop=(f == 2),
                                )
                        else:  # host_t / strided: single K=24 matmul
                            nc.tensor.matmul(
                                z[:, c * f_cols:(c + 1) * f_cols],
                                lhsT=w0t_sb[:],
                                rhs=xss[g][:, c * f_cols:(c + 1) * f_cols],
                                start=True, stop=True,
                            )
                for g in gs:
                    h = hpool.tile([128, g_cols], F32, tag="h")
                    nc.scalar.activation(h[:], zs[g][:], Tanh,
                                         bias=bias_sb[:, 0:1])
                    hs[g] = h

                # ---- hidden layers
                for l in range(N_HIDDEN):
                    for g in gs:
                        z = zpool.tile([128, g_cols], F32, tag="z")
                        zs[g] = z
                        for c in range(cpg):
                            nc.tensor.matmul(
                                z[:, c * f_cols:(c + 1) * f_cols],
                                lhsT=wh_sb[:, l * 128:(l + 1) * 128],
                                rhs=hs[g][:, c * f_cols:(c + 1) * f_cols],
                                start=True, stop=True,
                            )
                    for g in gs:
                        h2 = hpool.tile([128, g_cols], F32, tag="h")
                        nc.scalar.activation(h2[:], zs[g][:], Tanh,
                                             bias=bias_sb[:, l + 1:l + 2])
                        hs[g] = h2

                # ---- output layer -> [S, g_cols]
                for g in gs:
                    zo = zpool.tile([S, g_cols], F32, tag="z")
                    zs[g] = zo
                    for c in range(cpg):
                        nc.tensor.matmul(
                            zo[:, c * f_cols:(c + 1) * f_cols],
                            lhsT=wo_sb[:],
                            rhs=hs[g][:, c * f_cols:(c + 1) * f_cols],
                            start=True, stop=True,
                        )
                for g in gs:
                    ys = ypool.tile([S, g_cols], F32, tag="ys")
                    nc.scalar.activation(ys[:], zs[g][:], Sigmoid,
                                         bias=bias_sb[0:S, 9:10])
                    nc.sync.dma_start(yr[:, g * g_cols:(g + 1) * g_cols],
                                      ys[:])

    nc.compile()
    return nc


# ---------------------------------------------------------------------------
# Tile-position variant: 32 streams in 16 pairs, each layer = one "wave" of
# 16 CONCURRENT 32x32 tile matmuls (measured ~3.4x the full-array fp32 rate,
# since a 32x32 tile holding blockdiag(W.T x2) does 50% useful MACs vs the
# 128x128 block-diagonal's 12.5%).  Pair p lives at coordinates
# (strip s, free-block fb) of the current [128, 4F] activation tile; the MM
# for a pair is tile_position (32*s, 32*s') with s' = (s+fb) % 4, writing
# PSUM (strip s', free-block s) -- so positions evolve by the invertible map
# (s, fb) -> ((s+fb)%4, s) and all 16 (row, col) tiles are used exactly once
# per wave.  PE drops to ~1 us per 8192-point wave; the scalar engine's
# tanh throughput becomes the bottleneck.
# ---------------------------------------------------------------------------

N_STREAMS = 32
N_PAIRS = 16
L32 = N_CORE // N_STREAMS    # 8192 points per stream


def _pair_positions():
    """pos[layer][p] = (strip, freeblock) for layers 1..10 (post-L0..output)."""
    pos = [{p: (p // 4, p % 4) for p in range(N_PAIRS)}]
    for _ in range(N_HIDDEN + 1):
        nxt = {}
        for p, (s, fb) in pos[-1].items():
            nxt[p] = ((s + fb) % 4, s)
        pos.append(nxt)
    return pos


def format_inputs_tiles(W0, b0, Wh, bh, Wo, bo):
    W0 = np.asarray(W0, np.float32)
    b0 = np.asarray(b0, np.float32)
    Wh = np.asarray(Wh, np.float32)
    bh = np.asarray(bh, np.float32)
    Wo = np.asarray(Wo, np.float32)
    bo = np.asarray(bo, np.float32)

    # L0 stationary [128, 32]: strip c rows 3*sl+f, cols 16*sl+m = W0[m, f]
    w0t32 = np.zeros((128, 32), np.float32)
    for c in range(4):
        for sl in range(2):
            for f in range(3):
                w0t32[32 * c + 3 * sl + f, 16 * sl:16 * sl + W] = W0[:, f]

    # hidden stationaries [128, 32*8]: strip c = blockdiag(Wh[l].T x2)
    wh32 = np.zeros((128, 32 * N_HIDDEN), np.float32)
    for l in range(N_HIDDEN):
        for c in range(4):
            for sl in range(2):
                r = 32 * c + 16 * sl
                wh32[r:r + W, 32 * l + 16 * sl:32 * l + 16 * sl + W] = Wh[l].T

    # output stationary [128, 32]: strip c rows 16*sl+fi, col sl = Wo[0, fi];
    # cols 2..31 zero so the MM writes its full 32-row strip (cost is
    # per-column, so the padding is free and keeps PSUM fully initialized)
    wo32 = np.zeros((128, 32), np.float32)
    for c in range(4):
        for sl in range(2):
            wo32[32 * c + 16 * sl:32 * c + 16 * sl + W, sl] = Wo[0, :]

    # bias table [128, 10]: tanh cols use rows 32c+16sl+m; sigmoid col 9
    # uses rows 32c+sl
    bias32 = np.zeros((128, 10), np.float32)
    for c in range(4):
        for sl in range(2):
            r = 32 * c + 16 * sl
            bias32[r:r + W, 0] = b0
            for l in range(N_HIDDEN):
                bias32[r:r + W, 1 + l] = bh[l]
    bias32[:, 9] = bo[0]
    for c in range(4):
        for sl in range(2):
            pass

    # dense output stationary [128, 8] for tiles2: rhs strip s, slot sl
    # -> out row 2s+sl (one full-array MM replaces 16 tile MMs)
    wod = np.zeros((128, 8), np.float32)
    for s in range(4):
        for sl in range(2):
            wod[32 * s + 16 * sl:32 * s + 16 * sl + W, 2 * s + sl] = Wo[0, :]
    return {"w0t32": w0t32, "wh32": wh32, "wo32": wo32, "bias32": bias32,
            "wod": wod}


def host_pack_x_tiles(x_core, f_cols=512):
    """[n_core, 3] -> packed [24, l32*4]: per wave w, strip b rows 6b+3sl+f,
    free w*4F + a*F + n = x[(2*(4a+b)+sl)*l32 + w*F + n, f]."""
    l32 = x_core.shape[0] // N_STREAMS
    nw = l32 // f_cols
    # [a, b, sl, w, n, f] -> [b, sl, f, w, a, n]
    xv = np.asarray(x_core, np.float32).reshape(4, 4, 2, nw, f_cols, 3)
    out = xv.transpose(1, 2, 5, 3, 0, 4)
    return np.ascontiguousarray(out).reshape(24, nw * 4 * f_cols)


def host_unpack_y_tiles(y_raw, f_cols=512):
    """y_raw [128, l32*4] -> y [n_core, 1] using final pair positions."""
    l32 = y_raw.shape[1] // 4
    nw = l32 // f_cols
    pos_out = _pair_positions()[N_HIDDEN + 1]
    yv = y_raw.reshape(8, nw, 4, f_cols)
    rows = np.array([2 * pos_out[p][0] + sl
                     for p in range(N_PAIRS) for sl in range(2)])
    fbs = np.array([pos_out[p][1] for p in range(N_PAIRS)]).repeat(2)
    y = yv[rows, :, fbs, :]          # [32, nw, f_cols]
    return np.ascontiguousarray(y).reshape(N_STREAMS * l32, 1)


def host_unpack_y_tiles2(y_raw, f_cols=512):
    """dense-Lout y_raw [8*nw, 4F]: row 8w+2s+sl, col fb*F+n'."""
    nw = y_raw.shape[0] // 8
    pos8 = _pair_positions()[N_HIDDEN]
    yv = y_raw.reshape(nw, 4, 2, 4, f_cols)   # [w, s, sl, fb, n']
    l32 = nw * f_cols
    y = np.empty((N_STREAMS, nw, f_cols), np.float32)
    for p in range(N_PAIRS):
        s, fb = pos8[p]
        for sl in range(2):
            y[2 * p + sl] = yv[:, s, sl, fb, :]
    return np.ascontiguousarray(y).reshape(N_STREAMS * l32, 1)


def build_program_tiles(n_core=N_CORE, f_cols=512, repeat=1,
                        num_devices=N_CORES, depth=2, xbufs=None, hbufs=None,
                        ybufs=None, dense_lout=False):
    """Tile-position wave kernel.  One wave = 16 concurrent 32x32 MMs
    (f_cols columns each) into a [128, 4*f_cols] PSUM tile (2 banks at
    f_cols=256), one ACT pass per wave-layer.  `depth` wave-chains are
    interleaved so ACT stays busy while the other chain's MMs run."""
    l32 = n_core // N_STREAMS
    nw = l32 // f_cols
    F = f_cols
    pos = _pair_positions()

    nc = bacc.Bacc("TRN2", target_bir_lowering=False, debug=False,
                   num_devices=num_devices)
    xt_ap = nc.dram_tensor("xt", [24, l32 * 4], F32, kind="ExternalInput").ap()
    w0_ap = nc.dram_tensor("w0t32", [128, 32], F32, kind="ExternalInput").ap()
    wh_ap = nc.dram_tensor("wh32", [128, 32 * N_HIDDEN], F32,
                           kind="ExternalInput").ap()
    wo_ap = nc.dram_tensor("wo32", [128, 32], F32, kind="ExternalInput").ap()
    wod_ap = nc.dram_tensor("wod", [128, 8], F32, kind="ExternalInput").ap()
    bias_ap = nc.dram_tensor("bias32", [128, 10], F32,
                             kind="ExternalInput").ap()
    # only rows 32c+sl of the sigmoid output carry data; ship them packed
    if dense_lout:
        y_ap = nc.dram_tensor("y_raw", [8 * nw, 4 * f_cols], F32,
                              kind="ExternalOutput").ap()
    else:
        y_ap = nc.dram_tensor("y_raw", [8, l32 * 4], F32,
                              kind="ExternalOutput").ap()

    Tanh = mybir.ActivationFunctionType.Tanh
    Sigmoid = mybir.ActivationFunctionType.Sigmoid

    with tile.TileContext(nc) as tc, ExitStack() as ctx:
        consts = ctx.enter_context(tc.tile_pool(name="consts", bufs=1))
        xpool = ctx.enter_context(tc.tile_pool(
            name="xpool", bufs=xbufs or depth + 1))
        hpool = ctx.enter_context(tc.tile_pool(
            name="hpool", bufs=hbufs or 2 * depth + 1))
        ypool = ctx.enter_context(tc.tile_pool(
            name="ypool", bufs=ybufs or depth + 1))
        # one PSUM slot is [128, 4F] with free-block i = one full bank at
        # F=512, so concurrent tile drains never share a (bank, partition)
        zbufs = max(2, 8 // max(1, (4 * f_cols * 4) // 2048))
        zpool = ctx.enter_context(tc.tile_pool(name="zpool", bufs=zbufs,
                                               space="PSUM"))

        w0_sb = consts.tile([128, 32], F32)
        nc.sync.dma_start(w0_sb[:], w0_ap[:])
        wh_sb = consts.tile([128, 32 * N_HIDDEN], F32)
        nc.sync.dma_start(wh_sb[:], wh_ap[:])
        wo_sb = consts.tile([128, 32], F32)
        nc.sync.dma_start(wo_sb[:], wo_ap[:])
        wod_sb = consts.tile([128, 8], F32)
        nc.sync.dma_start(wod_sb[:], wod_ap[:])
        bias_sb = consts.tile([128, 10], F32)
        nc.sync.dma_start(bias_sb[:], bias_ap[:])

        for _rep in range(repeat):
            if dense_lout:
                ystage = ypool.tile([8 * nw, 4 * f_cols], F32, tag="ystage",
                                    bufs=1)
            for w0i in range(0, nw, depth):
                ws = range(w0i, min(w0i + depth, nw))
                xss, hs, zs = {}, {}, {}
                for wv in ws:
                    xs = xpool.tile([128, 4 * F], F32, tag="xs")
                    for b in range(4):
                        nc.sync.dma_start(
                            xs[32 * b:32 * b + 6, :],
                            xt_ap[6 * b:6 * b + 6,
                                  wv * 4 * F:(wv + 1) * 4 * F])
                    xss[wv] = xs

                # ---- layer 0 wave: pair (a,b): tile (b, a), rhs strip b
                # fb a, out (strip a, fb b)
                for wv in ws:
                    z = zpool.tile([128, 4 * F], F32, tag="z")
                    zs[wv] = z
                    for p in range(N_PAIRS):
                        a, b = p // 4, p % 4
                        nc.tensor.matmul(
                            z[32 * a:32 * (a + 1), b * F:(b + 1) * F],
                            lhsT=w0_sb[32 * b:32 * b + 6, :],
                            rhs=xss[wv][32 * b:32 * b + 6,
                                        a * F:(a + 1) * F],
                            start=True, stop=True,
                            tile_position=(32 * b, 32 * a),
                            skip_group_check=True,
                        )
                for wv in ws:
                    h = hpool.tile([128, 4 * F], F32, tag="h")
                    nc.scalar.activation(h[:], zs[wv][:], Tanh,
                                         bias=bias_sb[:, 0:1])
                    hs[wv] = h

                # ---- hidden waves
                for l in range(N_HIDDEN):
                    cur, nxt = pos[l], pos[l + 1]
                    for wv in ws:
                        z = zpool.tile([128, 4 * F], F32, tag="z")
                        zs[wv] = z
                        for p in range(N_PAIRS):
                            s, fb = cur[p]
                            s2 = nxt[p][0]
                            nc.tensor.matmul(
                                z[32 * s2:32 * (s2 + 1), s * F:(s + 1) * F],
                                lhsT=wh_sb[32 * s:32 * (s + 1),
                                           32 * l:32 * (l + 1)],
                                rhs=hs[wv][32 * s:32 * (s + 1),
                                           fb * F:(fb + 1) * F],
                                start=True, stop=True,
                                tile_position=(32 * s, 32 * s2),
                                skip_group_check=True,
                            )
                    for wv in ws:
                        h2 = hpool.tile([128, 4 * F], F32, tag="h")
                        nc.scalar.activation(h2[:], zs[wv][:], Tanh,
                                             bias=bias_sb[:, l + 1:l + 2])
                        hs[wv] = h2

                # ---- output wave
                cur, nxt = pos[N_HIDDEN], pos[N_HIDDEN + 1]
                if dense_lout:
                    # one full-array MM per f_cols chunk -> dense [8, 4F]
                    for wv in ws:
                        zo = zpool.tile([128, 4 * F], F32, tag="z")
                        zs[wv] = zo
                        for c in range(4):
                            nc.tensor.matmul(
                                zo[0:8, c * F:(c + 1) * F],
                                lhsT=wod_sb[:],
                                rhs=hs[wv][:, c * F:(c + 1) * F],
                                start=True, stop=True,
                            )
                    for wv in ws:
                        tmp = ypool.tile([8, 4 * F], F32, tag="ys")
                        nc.vector.tensor_copy(tmp[:], zs[wv][0:8, :])
                        nc.sync.dma_start(
                            ystage[8 * wv:8 * wv + 8, :], tmp[:])
                else:
                    for wv in ws:
                        zo = zpool.tile([128, 4 * F], F32, tag="z")
                        zs[wv] = zo
                        for p in range(N_PAIRS):
                            s, fb = cur[p]
                            s2 = nxt[p][0]
                            nc.tensor.matmul(
                                zo[32 * s2:32 * (s2 + 1), s * F:(s + 1) * F],
                                lhsT=wo_sb[32 * s:32 * (s + 1), :],
                                rhs=hs[wv][32 * s:32 * (s + 1),
                                           fb * F:(fb + 1) * F],
                                start=True, stop=True,
                                tile_position=(32 * s, 32 * s2),
                                skip_group_check=True,
                            )
                    for wv in ws:
                        ys = ypool.tile([128, 4 * F], F32, tag="ys")
                        nc.scalar.activation(ys[:], zs[wv][:], Sigmoid,
                                             bias=bias_sb[:, 9:10])
                        for c in range(4):
                            nc.sync.dma_start(
                                y_ap[2 * c:2 * c + 2,
                                     wv * 4 * F:(wv + 1) * 4 * F],
                                ys[32 * c:32 * c + 2, :])

            if dense_lout:
                ysig = ypool.tile([8 * nw, 4 * f_cols], F32, tag="ysig",
                                  bufs=1)
                nc.scalar.activation(ysig[:], ystage[:], Sigmoid,
                                     bias=bias_sb[0:8 * nw, 9:10])
                nc.sync.dma_start(y_ap[:], ysig[:])

    nc.compile()
    return nc


# ---------------------------------------------------------------------------
# v2 "pairsig": tiles kernel with pair-dense output + DMA queue separation.
#
# Changes vs build_program_tiles (measured 396 us; ACT busy only ~283 us):
#  1. All DMAs shared the in-order SP queue, so each pair's x prefetch queued
#     behind sigmoid-gated y stores => ~13 us ACT bubble per pair.  Now x
#     loads alternate SP/DVE queues and y stores ride the GPSIMD queue.
#  2. Output layer: the two waves of a pair accumulate into one [16, 4F]
#     PSUM tile via two full-array float32r matmuls with zero-padded
#     stationaries (wodA | wodB), so sigmoid runs once per pair (8 ACT
#     instrs instead of 16).  f32r's 2^-13 noise is fine on the final layer
#     (no amplification through later layers).
#  3. Pair p's L0 matmuls are emitted BEFORE pair p-1's output matmuls so
#     the PE always has the next z ready when ACT frees a PSUM slot.
# ACT floor: (144 tanh + 8 sigmoid) x ~1.78 us ~= 270 us/core.
# ---------------------------------------------------------------------------


def format_inputs_tiles2(W0, b0, Wh, bh, Wo, bo):
    base = format_inputs_tiles(W0, b0, Wh, bh, Wo, bo)
    Wo = np.asarray(Wo, np.float32)
    # Pair-dense output stationaries [128, 16]: chain A -> out rows 0..7,
    # chain B -> rows 8..15; row index r = 2s+sl contracts h rows 32s+16sl.
    wodA = np.zeros((128, 16), np.float32)
    wodB = np.zeros((128, 16), np.float32)
    for s in range(4):
        for sl in range(2):
            r0 = 32 * s + 16 * sl
            wodA[r0:r0 + W, 2 * s + sl] = Wo[0, :]
            wodB[r0:r0 + W, 8 + 2 * s + sl] = Wo[0, :]
    base["wodA"] = round_f32r(wodA)
    base["wodB"] = round_f32r(wodB)
    return base


def host_unpack_y_pairsig(y_raw, f_cols=512):
    """y_raw [128, 4F]: row 16p + 8*ch + 2s+sl, col fb*F+n holds the output
    of pair pp (pos8[pp] = (s, fb)), slot sl, wave w = 2p+ch, point n."""
    F = f_cols
    npair = y_raw.shape[0] // 16
    pos8 = _pair_positions()[N_HIDDEN]
    yv = y_raw.reshape(npair, 2, 4, 2, 4, F)    # [p, ch, s, sl, fb, n]
    nwav = 2 * npair
    l32 = nwav * F
    y = np.empty((N_STREAMS, nwav, F), np.float32)
    for pp in range(N_PAIRS):
        s, fb = pos8[pp]
        for sl in range(2):
            y[2 * pp + sl] = yv[:, :, s, sl, fb, :].reshape(nwav, F)
    return np.ascontiguousarray(y).reshape(N_STREAMS * l32, 1)


def build_program_tiles2(n_core=N_CORE, f_cols=512, repeat=1,
                         num_devices=N_CORES, xbufs=5, hbufs=7, ybufs=3,
                         out_f32r=True):
    l32 = n_core // N_STREAMS
    nw = l32 // f_cols
    F = f_cols
    assert nw % 2 == 0
    pos = _pair_positions()

    nc = bacc.Bacc("TRN2", target_bir_lowering=False, debug=False,
                   num_devices=num_devices)
    xt_ap = nc.dram_tensor("xt", [24, l32 * 4], F32, kind="ExternalInput").ap()
    w0_ap = nc.dram_tensor("w0t32", [128, 32], F32, kind="ExternalInput").ap()
    wh_ap = nc.dram_tensor("wh32", [128, 32 * N_HIDDEN], F32,
                           kind="ExternalInput").ap()
    wodA_ap = nc.dram_tensor("wodA", [128, 16], F32R,
                             kind="ExternalInput").ap()
    wodB_ap = nc.dram_tensor("wodB", [128, 16], F32R,
                             kind="ExternalInput").ap()
    bias_ap = nc.dram_tensor("bias32", [128, 10], F32,
                             kind="ExternalInput").ap()
    y_ap = nc.dram_tensor("y_raw", [8 * nw, 4 * F], F32,
                          kind="ExternalOutput").ap()

    Tanh = mybir.ActivationFunctionType.Tanh
    Sigmoid = mybir.ActivationFunctionType.Sigmoid
    xq = [nc.sync, nc.sync]         # x loads: SP HWDGE queue only
    yq = nc.gpsimd                  # y stores: SWDGE, off the x path

    with tile.TileContext(nc) as tc, ExitStack() as ctx:
        consts = ctx.enter_context(tc.tile_pool(name="consts", bufs=1))
        xpool = ctx.enter_context(tc.tile_pool(name="xpool", bufs=xbufs))
        hpool = ctx.enter_context(tc.tile_pool(name="hpool", bufs=hbufs))
        ypool = ctx.enter_context(tc.tile_pool(name="ypool", bufs=ybufs))
        zpool = ctx.enter_context(tc.tile_pool(name="zpool", bufs=2,
                                               space="PSUM"))

        w0_sb = consts.tile([128, 32], F32)
        nc.sync.dma_start(w0_sb[:], w0_ap[:])
        wh_sb = consts.tile([128, 32 * N_HIDDEN], F32)
        nc.sync.dma_start(wh_sb[:], wh_ap[:])
        wodA_sb = consts.tile([128, 16], F32R)
        nc.sync.dma_start(wodA_sb[:], wodA_ap[:])
        wodB_sb = consts.tile([128, 16], F32R)
        nc.sync.dma_start(wodB_sb[:], wodB_ap[:])
        bias_sb = consts.tile([128, 10], F32)
        nc.sync.dma_start(bias_sb[:], bias_ap[:])

        def load_x(wv):
            xs = xpool.tile([128, 4 * F], F32, tag="xs")
            q = xq[wv % 2]
            for b in range(4):
                q.dma_start(
                    xs[32 * b:32 * b + 6, :],
                    xt_ap[6 * b:6 * b + 6, wv * 4 * F:(wv + 1) * 4 * F])
            return xs

        def l0_wave(xs):
            z = zpool.tile([128, 4 * F], F32, tag="z")
            for p in range(N_PAIRS):
                a, b = p // 4, p % 4
                nc.tensor.matmul(
                    z[32 * a:32 * (a + 1), b * F:(b + 1) * F],
                    lhsT=w0_sb[32 * b:32 * b + 6, :],
                    rhs=xs[32 * b:32 * b + 6, a * F:(a + 1) * F],
                    start=True, stop=True,
                    tile_position=(32 * b, 32 * a),
                    skip_group_check=True,
                )
            return z

        def hidden_wave(h, l):
            cur, nxt = pos[l], pos[l + 1]
            z = zpool.tile([128, 4 * F], F32, tag="z")
            for p in range(N_PAIRS):
                s, fb = cur[p]
                s2 = nxt[p][0]
                nc.tensor.matmul(
                    z[32 * s2:32 * (s2 + 1), s * F:(s + 1) * F],
                    lhsT=wh_sb[32 * s:32 * (s + 1), 32 * l:32 * (l + 1)],
                    rhs=h[32 * s:32 * (s + 1), fb * F:(fb + 1) * F],
                    start=True, stop=True,
                    tile_position=(32 * s, 32 * s2),
                    skip_group_check=True,
                )
            return z

        def act(z, col, tag="h", dtype=F32):
            h = hpool.tile([128, 4 * F], dtype, tag=tag)
            nc.scalar.activation(h[:], z[:], Tanh,
                                 bias=bias_sb[:, col:col + 1])
            return h

        def out_mms(hA, hB):
            """Pair-dense output: accumulate both chains into [16, 4F].
            hA/hB are F32R (rounded on ACT write), so the MMs run f32r
            (1 cyc/row vs fp32's 4) and stay off the boundary critical path.
            """
            zo = zpool.tile([16, 4 * F], F32, tag="z")
            for c in range(4):
                for lhs, h, st in ((wodA_sb, hA, True), (wodB_sb, hB, False)):
                    nc.tensor.matmul(
                        zo[0:16, c * F:(c + 1) * F],
                        lhsT=lhs[:], rhs=h[:, c * F:(c + 1) * F],
                        start=st, stop=not st,
                    )
            return zo

        def sig_out(zo, p):
            ys = ypool.tile([16, 4 * F], F32, tag="ys")
            nc.scalar.activation(ys[:], zo[:], Sigmoid,
                                 bias=bias_sb[0:16, 9:10])
            yq.dma_start(y_ap[16 * p:16 * p + 16, :], ys[:])

        for _rep in range(repeat):
            prev = None     # (zo emitted?, hA8, hB8, pair_idx)
            for p in range(nw // 2):
                wA, wB = 2 * p, 2 * p + 1
                xsA = load_x(wA)
                xsB = load_x(wB)
                zA = l0_wave(xsA)
                if prev is not None:
                    zo = out_mms(prev[0], prev[1])
                hA = act(zA, 0)
                if prev is not None:
                    sig_out(zo, prev[2])
                zB = l0_wave(xsB)
                hB = act(zB, 0)
                for l in range(N_HIDDEN):
                    # the last hidden tanh feeds only the f32r output MMs;
                    # write it pre-rounded to the f32r grid
                    dt = F32R if (out_f32r and l == N_HIDDEN - 1) else F32
                    zA = hidden_wave(hA, l)
                    hA = act(zA, l + 1, dtype=dt)
                    zB = hidden_wave(hB, l)
                    hB = act(zB, l + 1, dtype=dt)
                prev = (hA, hB, p)
            zo = out_mms(prev[0], prev[1])
            sig_out(zo, prev[2])

    nc.compile()
    return nc


_RUNNER_CACHE = {}
L0_MODE = "host_t"
KERNEL_MODE = "tiles2"  # "tiles2" | "tiles" | "block"


def host_transpose_x(x_core):
    """[n_core, 3] -> [24, n_core/S]: partition 8f+j = feature f of stream j."""
    L = x_core.shape[0] // S
    return np.ascontiguousarray(
        x_core.reshape(S, L, 3).transpose(2, 0, 1).reshape(24, L))


def make_in_maps(x, consts, l0_mode=None):
    """Per-core input maps from the full x [N_FULL, 3] + formatted weights."""
    l0_mode = l0_mode or L0_MODE
    in_maps = []
    for c in range(N_CORES):
        xc = x[c * N_CORE:(c + 1) * N_CORE]
        if l0_mode == "host_t":
            m = {"xt": host_transpose_x(xc)}
        else:
            m = {"x": np.ascontiguousarray(xc)}
        m.update(consts)
        in_maps.append(m)
    return in_maps


def make_runner(nc, n_cores=N_CORES):
    """Build a reusable jitted PJRT runner for the SPMD program (mirrors
    bass2jax.run_bass_via_pjrt's multi-core path, minus output donation, so
    the NEFF compile is paid once and later calls are just execution)."""
    import jax
    from jax.sharding import Mesh, PartitionSpec, NamedSharding
    from jax.experimental.shard_map import shard_map
    from concourse import bass2jax

    bass2jax.install_neuronx_cc_hook()
    partition_name = (nc.partition_id_tensor.name
                      if nc.partition_id_tensor else None)
    in_names, out_names, out_avals = [], [], []
    for alloc in nc.m.functions[0].allocations:
        if not isinstance(alloc, mybir.MemoryLocationSet):
            continue
        name = alloc.memorylocations[0].name
        if alloc.kind == "ExternalInput":
            if name != partition_name:
                in_names.append(name)
        elif alloc.kind == "ExternalOutput":
            out_names.append(name)
            out_avals.append(jax.core.ShapedArray(
                tuple(alloc.tensor_shape), mybir.dt.np(alloc.dtype)))
    n_params = len(in_names)
    all_in_names = list(in_names) + list(out_names)
    if partition_name is not None:
        all_in_names.append(partition_name)

    def _body(*args):
        operands = list(args)
        if partition_name is not None:
            operands.append(bass2jax.partition_id_tensor())
        outs = bass2jax._bass_exec_p.bind(
            *operands,
            out_avals=tuple(out_avals),
            in_names=tuple(all_in_names),
            out_names=tuple(out_names),
            lowering_input_output_aliases=(),
            sim_require_finite=True,
            sim_require_nnan=True,
            nc=nc,
        )
        return tuple(outs)

    devices = jax.devices()[:n_cores]
    mesh = Mesh(np.asarray(devices), ("core",))
    n_outs = len(out_names)
    in_specs = (PartitionSpec("core"),) * (n_params + n_outs)
    out_specs = (PartitionSpec("core"),) * n_outs
    fn = jax.jit(shard_map(_body, mesh=mesh, in_specs=in_specs,
                           out_specs=out_specs, check_rep=False),
                 keep_unused=True)
    sharding = NamedSharding(mesh, PartitionSpec("core"))

    def prepare(in_maps):
        concat_in = [
            np.concatenate([np.asarray(in_maps[c][n])
                            for c in range(n_cores)], axis=0)
            for n in in_names
        ]
        concat_zero = [
            np.zeros((n_cores * a.shape[0],) + tuple(a.shape[1:]), a.dtype)
            for a in out_avals
        ]
        return [jax.device_put(a, sharding) for a in concat_in + concat_zero]

    return fn, prepare, out_names


def _get_runner(key=(N_CORE, 1024, 512, 1, L0_MODE)):
    if key not in _RUNNER_CACHE:
        n_core, g_cols, f_cols, repeat, l0_mode = key
        nc = build_program(n_core, g_cols, f_cols, repeat, l0_mode=l0_mode)
        _RUNNER_CACHE[key] = make_runner(nc)
    return _RUNNER_CACHE[key]


def _get_runner_tiles(key=(N_CORE, 512, 1)):
    if key not in _RUNNER_CACHE:
        n_core, f_cols, repeat = key
        nc = build_program_tiles(n_core, f_cols, repeat)
        _RUNNER_CACHE[key] = make_runner(nc)
    return _RUNNER_CACHE[key]


def _get_runner_tiles2(key=(N_CORE, 512, 1)):
    if ("t2",) + key not in _RUNNER_CACHE:
        n_core, f_cols, repeat = key
        nc = build_program_tiles2(n_core, f_cols, repeat)
        _RUNNER_CACHE[("t2",) + key] = make_runner(nc)
    return _RUNNER_CACHE[("t2",) + key]


def kernel(x, W0, b0, Wh, bh, Wo, bo):
    import jax
    x = np.asarray(x, np.float32)
    assert x.shape == (N_FULL, 3), x.shape
    if KERNEL_MODE == "tiles2":
        consts = format_inputs_tiles2(W0, b0, Wh, bh, Wo, bo)
        fn, prepare, out_names = _get_runner_tiles2()
        in_maps = []
        for c in range(N_CORES):
            m = {"xt": host_pack_x_tiles(x[c * N_CORE:(c + 1) * N_CORE])}
            m.update({k: consts[k] for k in
                      ("w0t32", "wh32", "wodA", "wodB", "bias32")})
            in_maps.append(m)
        args = prepare(in_maps)
        outs = fn(*args)
        jax.block_until_ready(outs)
        y_raw = np.asarray(outs[out_names.index("y_raw")])
        y_raw = y_raw.reshape(N_CORES, 128, -1)
        y = np.concatenate(
            [host_unpack_y_pairsig(y_raw[c]) for c in range(N_CORES)], axis=0)
        return np.ascontiguousarray(y.astype(np.float32))
    if KERNEL_MODE == "tiles":
        consts = format_inputs_tiles(W0, b0, Wh, bh, Wo, bo)
        fn, prepare, out_names = _get_runner_tiles()
        in_maps = []
        for c in range(N_CORES):
            m = {"xt": host_pack_x_tiles(x[c * N_CORE:(c + 1) * N_CORE])}
            m.update(consts)
            in_maps.append(m)
        args = prepare(in_maps)
        outs = fn(*args)
        jax.block_until_ready(outs)
        y_raw = np.asarray(outs[out_names.index("y_raw")])
        y_raw = y_raw.reshape(N_CORES, 8, L32 * 4)
        y = np.concatenate(
            [host_unpack_y_tiles(y_raw[c]) for c in range(N_CORES)], axis=0)
        return np.ascontiguousarray(y.astype(np.float32))
    consts = format_inputs(W0, b0, Wh, bh, Wo, bo)
    fn, prepare, out_names = _get_runner()
    args = prepare(make_in_maps(x, consts))
    outs = fn(*args)
    jax.block_until_ready(outs)
    y = np.asarray(outs[out_names.index("y")])
    return np.ascontiguousarray(y.reshape(N_FULL, 1).astype(np.float32))



# revision 17
# speedup vs baseline: 1.0340x; 1.0340x over previous
"""CPPN dense-MLP Trainium2 kernel.

Network (per point): 3 -> 16 (tanh) -> 8 x [16 -> 16 (tanh)] -> 1 (sigmoid).
2,097,152 points, pure data parallel across 8 NeuronCores.

Per-core layout: the core's 262,144 points are split into S=8 streams of
32,768 points.  Activations live in SBUF/PSUM "block layout": partition
16*j + m holds feature m of stream j, free dim indexes points within the
stream.  Every layer is then a single 128x128 block-diagonal stationary
matmul on the tensor engine (8 independent 16x16 matmuls per cycle).

Layer 0 (K=3) needs x with features on partitions; the host pre-transposes
each core's shard to [24, 32768] (partition 8f+j = feature f of stream j;
a cheap numpy reshape, part of sharding), so layer 0 is a single K=24
matmul with fully contiguous DMA loads.  (Alternative modes kept for A/B:
"three_mm" = stride-3 rhs APs over natural-layout x, ~+110us PE;
"strided" = DMA-side transpose, ~+600us of 4-byte-element descriptors.)

Activations (tanh / sigmoid + bias) run on the scalar engine directly from
PSUM into SBUF; tanh and sigmoid share one ACT table set so there are no
table reloads.

Matmuls are full float32 (4 cycles/column): this network doubles any
injected error every layer (~x250 over the 9-layer chain), so float32r's
~2^-13 per-product noise lands at ~0.3 absolute output error while fp32
stays at ~3e-5.  fp32 is mandatory for the gate, making the kernel
PE-bound.
"""

import numpy as np
import ml_dtypes
from contextlib import ExitStack


def round_f32r(a):
    """Round fp32 to the float32r grid (value representable as bf16 + bf16)."""
    a = np.asarray(a, np.float32)
    hi = a.astype(ml_dtypes.bfloat16).astype(np.float32)
    lo = (a - hi).astype(ml_dtypes.bfloat16).astype(np.float32)
    return hi + lo

import concourse.bass as bass
import concourse.tile as tile
from concourse import bacc, mybir
from concourse.bass_utils import run_bass_kernel_spmd

F32 = mybir.dt.float32
F32R = mybir.dt.float32r

N_FULL = 2097152
N_CORES = 8
N_CORE = N_FULL // N_CORES  # 262144 points per core
S = 8                       # streams per core
W = 16                      # hidden width
N_HIDDEN = 8


def format_inputs(W0, b0, Wh, bh, Wo, bo):
    """Build the block-diagonal stationary matrices + bias table (numpy)."""
    W0 = np.asarray(W0, np.float32)
    b0 = np.asarray(b0, np.float32)
    Wh = np.asarray(Wh, np.float32)
    bh = np.asarray(bh, np.float32)
    Wo = np.asarray(Wo, np.float32)
    bo = np.asarray(bo, np.float32)

    # Layer 0: three [S, 128] stationaries (one per input feature), packed
    # side by side into [S, 3*128].  stationary_f[j, 16j+m] = W0[m, f].
    w0f = np.zeros((S, 3 * 128), np.float32)
    for f in range(3):
        for j in range(S):
            w0f[j, f * 128 + 16 * j:f * 128 + 16 * j + W] = W0[:, f]

    # Hidden layers: [128, 8*128]; slice l is blockdiag(Wh[l].T x8):
    # stat[16j+fi, 16j+m] = Wh[l][m, fi].
    wh = np.zeros((128, N_HIDDEN * 128), np.float32)
    for l in range(N_HIDDEN):
        for j in range(S):
            r = 16 * j
            wh[r:r + W, l * 128 + r:l * 128 + r + W] = Wh[l].T

    # Output layer: [128, S]: stat[16j+fi, j] = Wo[0, fi].
    wo = np.zeros((128, S), np.float32)
    for j in range(S):
        wo[16 * j:16 * j + W, j] = Wo[0, :]

    # Bias table [128, 10]: col 0 = b0 block, cols 1..8 = bh blocks,
    # col 9 rows 0..7 = bo.
    bias = np.zeros((128, 10), np.float32)
    for j in range(S):
        bias[16 * j:16 * j + W, 0] = b0
        for l in range(N_HIDDEN):
            bias[16 * j:16 * j + W, 1 + l] = bh[l]
    bias[0:S, 9] = bo[0]

    # Layer 0 alternative: single [24, 128] stationary for K=24 matmul
    # over a DMA-transposed x (f-major partitions): w0t[8f+j, 16j+m] = W0[m, f].
    w0t = np.zeros((24, 128), np.float32)
    for j in range(S):
        for f in range(3):
            w0t[8 * f + j, 16 * j:16 * j + W] = W0[:, f]

    return {"w0f": w0f, "w0t": w0t, "wh": wh, "wo": wo, "bias": bias}


def build_program(n_core=N_CORE, g_cols=1024, f_cols=512, repeat=1,
                  num_devices=N_CORES, depth=2, xbufs=None, hbufs=None,
                  ybufs=None, zbufs=None, l0_mode="three_mm"):
    """Build + compile the per-core Bass program (SPMD: same on all cores).

    n_core: points per core.  g_cols: free-dim columns per group (pipeline
    granularity; one ACT instruction per layer per group).  f_cols: free-dim
    columns per matmul (<= 512, one PSUM bank).  repeat: run the whole
    kernel body this many times (for wall-clock timing; output idempotent).
    """
    stream_len = n_core // S
    ng = stream_len // g_cols
    assert stream_len % g_cols == 0 and g_cols % f_cols == 0
    cpg = g_cols // f_cols  # matmul chunks per group

    nc = bacc.Bacc("TRN2", target_bir_lowering=False, debug=False,
                   num_devices=num_devices)
    if l0_mode == "host_t":
        xt_ap = nc.dram_tensor("xt", [24, n_core // S], F32,
                               kind="ExternalInput").ap()
        x_ap = None
    else:
        x_ap = nc.dram_tensor("x", [n_core, 3], F32,
                              kind="ExternalInput").ap()
    w0f_ap = nc.dram_tensor("w0f", [S, 3 * 128], F32, kind="ExternalInput").ap()
    w0t_ap = nc.dram_tensor("w0t", [24, 128], F32, kind="ExternalInput").ap()
    wh_ap = nc.dram_tensor("wh", [128, N_HIDDEN * 128], F32,
                           kind="ExternalInput").ap()
    wo_ap = nc.dram_tensor("wo", [128, S], F32, kind="ExternalInput").ap()
    bias_ap = nc.dram_tensor("bias", [128, 10], F32, kind="ExternalInput").ap()
    y_ap = nc.dram_tensor("y", [n_core, 1], F32, kind="ExternalOutput").ap()

    # DRAM views: stream-major.  x[(j n) f] -> [S, stream_len*3] so each
    # partition's group slice is one contiguous run.
    if x_ap is not None:
        xr = x_ap.rearrange("(j n) f -> j (n f)", j=S)
        # transposed view for strided-DMA layer 0: [S, 3, stream_len]
        xt_dram = x_ap.rearrange("(j n) f -> j f n", j=S)
    yr = y_ap.rearrange("(j n) o -> j (n o)", j=S)

    Tanh = mybir.ActivationFunctionType.Tanh
    Sigmoid = mybir.ActivationFunctionType.Sigmoid

    with tile.TileContext(nc) as tc, ExitStack() as ctx:
        if zbufs is None:
            # slots are bank (2KB) granular; fill the 8 banks
            banks_per_slot = max(1, (g_cols * 4) // 2048)
            zbufs = max(2, 8 // banks_per_slot)
        if xbufs is None:
            xbufs = depth + 1
        if hbufs is None:
            hbufs = 2 * depth + 1
        if ybufs is None:
            ybufs = depth + 1
        consts = ctx.enter_context(tc.tile_pool(name="consts", bufs=1))
        xpool = ctx.enter_context(tc.tile_pool(name="xpool", bufs=xbufs))
        hpool = ctx.enter_context(tc.tile_pool(name="hpool", bufs=hbufs))
        ypool = ctx.enter_context(tc.tile_pool(name="ypool", bufs=ybufs))
        zpool = ctx.enter_context(
            tc.tile_pool(name="zpool", bufs=zbufs, space="PSUM"))

        w0f_sb = consts.tile([S, 3 * 128], F32)
        nc.sync.dma_start(w0f_sb[:], w0f_ap[:])
        w0t_sb = consts.tile([24, 128], F32)
        nc.sync.dma_start(w0t_sb[:], w0t_ap[:])
        wh_sb = consts.tile([128, N_HIDDEN * 128], F32)
        nc.sync.dma_start(wh_sb[:], wh_ap[:])
        wo_sb = consts.tile([128, S], F32)
        nc.sync.dma_start(wo_sb[:], wo_ap[:])
        bias_sb = consts.tile([128, 10], F32)
        nc.sync.dma_start(bias_sb[:], bias_ap[:])

        # Interleave `depth` groups at each layer step: within a group the
        # PE's layer l+1 strictly follows ACT of layer l, so a single group
        # serializes PE<->ACT.  Emitting layer l for D groups back-to-back
        # gives the PE work while ACT drains the other groups' PSUM tiles.
        for _rep in range(repeat):
            for g0 in range(0, ng, depth):
                gs = range(g0, min(g0 + depth, ng))
                xss, hs, zs = {}, {}, {}
                for g in gs:
                    if l0_mode == "three_mm":
                        # ---- load x chunk [S, 3*g_cols], contiguous/stream
                        xs = xpool.tile([S, 3 * g_cols], F32, tag="xs")
                        nc.sync.dma_start(
                            xs[:], xr[:, g * 3 * g_cols:(g + 1) * 3 * g_cols])
                        # stride-3 feature view: [S, g_cols, 3]
                        xss[g] = xs.rearrange("p (n f) -> p n f", f=3)
                    elif l0_mode == "host_t":
                        # ---- x pre-transposed on host: contiguous [24, g]
                        xs = xpool.tile([24, g_cols], F32, tag="xs")
                        nc.sync.dma_start(
                            xs[:], xt_ap[:, g * g_cols:(g + 1) * g_cols])
                        xss[g] = xs
                    else:
                        # ---- strided-DMA transpose load: [24, g_cols]
                        xs = xpool.tile([24, g_cols], F32, tag="xs")
                        for f in range(3):
                            nc.sync.dma_start(
                                xs[8 * f:8 * (f + 1), :],
                                xt_dram[:, f, g * g_cols:(g + 1) * g_cols])
                        xss[g] = xs

                # ---- layer 0
                for g in gs:
                    z = zpool.tile([128, g_cols], F32, tag="z")
                    zs[g] = z
                    for c in range(cpg):
                        if l0_mode == "three_mm":
                            for f in range(3):
                                nc.tensor.matmul(
                                    z[:, c * f_cols:(c + 1) * f_cols],
                                    lhsT=w0f_sb[:, f * 128:(f + 1) * 128],
                                    rhs=xss[g][:, c * f_cols:(c + 1) * f_cols,
                                               f],
                                    start=(f == 0), stop=(f == 2),
                                )
                        else:  # host_t / strided: single K=24 matmul
                            nc.tensor.matmul(
                                z[:, c * f_cols:(c + 1) * f_cols],
                                lhsT=w0t_sb[:],
                                rhs=xss[g][:, c * f_cols:(c + 1) * f_cols],
                                start=True, stop=True,
                            )
                for g in gs:
                    h = hpool.tile([128, g_cols], F32, tag="h")
                    nc.scalar.activation(h[:], zs[g][:], Tanh,
                                         bias=bias_sb[:, 0:1])
                    hs[g] = h

                # ---- hidden layers
                for l in range(N_HIDDEN):
                    for g in gs:
                        z = zpool.tile([128, g_cols], F32, tag="z")
                        zs[g] = z
                        for c in range(cpg):
                            nc.tensor.matmul(
                                z[:, c * f_cols:(c + 1) * f_cols],
                                lhsT=wh_sb[:, l * 128:(l + 1) * 128],
                                rhs=hs[g][:, c * f_cols:(c + 1) * f_cols],
                                start=True, stop=True,
                            )
                    for g in gs:
                        h2 = hpool.tile([128, g_cols], F32, tag="h")
                        nc.scalar.activation(h2[:], zs[g][:], Tanh,
                                             bias=bias_sb[:, l + 1:l + 2])
                        hs[g] = h2

                # ---- output layer -> [S, g_cols]
                for g in gs:
                    zo = zpool.tile([S, g_cols], F32, tag="z")
                    zs[g] = zo
                    for c in range(cpg):
                        nc.tensor.matmul(
                            zo[:, c * f_cols:(c + 1) * f_cols],
                            lhsT=wo_sb[:],
                            rhs=hs[g][:, c * f_cols:(c + 1) * f_cols],
                            start=True, stop=True,
                        )
                for g in gs:
                    ys = ypool.tile([S, g_cols], F32, tag="ys")
                    nc.scalar.activation(ys[:], zs[g][:], Sigmoid,
                                         bias=bias_sb[0:S, 9:10])
                    nc.sync.dma_start(yr[:, g * g_cols:(g + 1) * g_cols],
                                      ys[:])

    nc.compile()
    return nc


# ---------------------------------------------------------------------------
# Tile-position variant: 32 streams in 16 pairs, each layer = one "wave" of
# 16 CONCURRENT 32x32 tile matmuls (measured ~3.4x the full-array fp32 rate,
# since a 32x32 tile holding blockdiag(W.T x2) does 50% useful MACs vs the
# 128x128 block-diagonal's 12.5%).  Pair p lives at coordinates
# (strip s, free-block fb) of the current [128, 4F] activation tile; the MM
# for a pair is tile_position (32*s, 32*s') with s' = (s+fb) % 4, writing
# PSUM (strip s', free-block s) -- so positions evolve by the invertible map
# (s, fb) -> ((s+fb)%4, s) and all 16 (row, col) tiles are used exactly once
# per wave.  PE drops to ~1 us per 8192-point wave; the scalar engine's
# tanh throughput becomes the bottleneck.
# ---------------------------------------------------------------------------

N_STREAMS = 32
N_PAIRS = 16
L32 = N_CORE // N_STREAMS    # 8192 points per stream


def _pair_positions():
    """pos[layer][p] = (strip, freeblock) for layers 1..10 (post-L0..output)."""
    pos = [{p: (p // 4, p % 4) for p in range(N_PAIRS)}]
    for _ in range(N_HIDDEN + 1):
        nxt = {}
        for p, (s, fb) in pos[-1].items():
            nxt[p] = ((s + fb) % 4, s)
        pos.append(nxt)
    return pos


def format_inputs_tiles(W0, b0, Wh, bh, Wo, bo):
    W0 = np.asarray(W0, np.float32)
    b0 = np.asarray(b0, np.float32)
    Wh = np.asarray(Wh, np.float32)
    bh = np.asarray(bh, np.float32)
    Wo = np.asarray(Wo, np.float32)
    bo = np.asarray(bo, np.float32)

    # L0 stationary [128, 32]: strip c rows 3*sl+f, cols 16*sl+m = W0[m, f]
    w0t32 = np.zeros((128, 32), np.float32)
    for c in range(4):
        for sl in range(2):
            for f in range(3):
                w0t32[32 * c + 3 * sl + f, 16 * sl:16 * sl + W] = W0[:, f]

    # hidden stationaries [128, 32*8]: strip c = blockdiag(Wh[l].T x2)
    wh32 = np.zeros((128, 32 * N_HIDDEN), np.float32)
    for l in range(N_HIDDEN):
        for c in range(4):
            for sl in range(2):
                r = 32 * c + 16 * sl
                wh32[r:r + W, 32 * l + 16 * sl:32 * l + 16 * sl + W] = Wh[l].T

    # output stationary [128, 32]: strip c rows 16*sl+fi, col sl = Wo[0, fi];
    # cols 2..31 zero so the MM writes its full 32-row strip (cost is
    # per-column, so the padding is free and keeps PSUM fully initialized)
    wo32 = np.zeros((128, 32), np.float32)
    for c in range(4):
        for sl in range(2):
            wo32[32 * c + 16 * sl:32 * c + 16 * sl + W, sl] = Wo[0, :]

    # bias table [128, 10]: tanh cols use rows 32c+16sl+m; sigmoid col 9
    # uses rows 32c+sl
    bias32 = np.zeros((128, 10), np.float32)
    for c in range(4):
        for sl in range(2):
            r = 32 * c + 16 * sl
            bias32[r:r + W, 0] = b0
            for l in range(N_HIDDEN):
                bias32[r:r + W, 1 + l] = bh[l]
    bias32[:, 9] = bo[0]
    for c in range(4):
        for sl in range(2):
            pass

    # dense output stationary [128, 8] for tiles2: rhs strip s, slot sl
    # -> out row 2s+sl (one full-array MM replaces 16 tile MMs)
    wod = np.zeros((128, 8), np.float32)
    for s in range(4):
        for sl in range(2):
            wod[32 * s + 16 * sl:32 * s + 16 * sl + W, 2 * s + sl] = Wo[0, :]
    return {"w0t32": w0t32, "wh32": wh32, "wo32": wo32, "bias32": bias32,
            "wod": wod}


def host_pack_x_tiles(x_core, f_cols=512):
    """[n_core, 3] -> packed [24, l32*4]: per wave w, strip b rows 6b+3sl+f,
    free w*4F + a*F + n = x[(2*(4a+b)+sl)*l32 + w*F + n, f]."""
    l32 = x_core.shape[0] // N_STREAMS
    nw = l32 // f_cols
    # [a, b, sl, w, n, f] -> [b, sl, f, w, a, n]
    xv = np.asarray(x_core, np.float32).reshape(4, 4, 2, nw, f_cols, 3)
    out = xv.transpose(1, 2, 5, 3, 0, 4)
    return np.ascontiguousarray(out).reshape(24, nw * 4 * f_cols)


def host_unpack_y_tiles(y_raw, f_cols=512):
    """y_raw [128, l32*4] -> y [n_core, 1] using final pair positions."""
    l32 = y_raw.shape[1] // 4
    nw = l32 // f_cols
    pos_out = _pair_positions()[N_HIDDEN + 1]
    yv = y_raw.reshape(8, nw, 4, f_cols)
    rows = np.array([2 * pos_out[p][0] + sl
                     for p in range(N_PAIRS) for sl in range(2)])
    fbs = np.array([pos_out[p][1] for p in range(N_PAIRS)]).repeat(2)
    y = yv[rows, :, fbs, :]          # [32, nw, f_cols]
    return np.ascontiguousarray(y).reshape(N_STREAMS * l32, 1)


def host_unpack_y_tiles2(y_raw, f_cols=512):
    """dense-Lout y_raw [8*nw, 4F]: row 8w+2s+sl, col fb*F+n'."""
    nw = y_raw.shape[0] // 8
    pos8 = _pair_positions()[N_HIDDEN]
    yv = y_raw.reshape(nw, 4, 2, 4, f_cols)   # [w, s, sl, fb, n']
    l32 = nw * f_cols
    y = np.empty((N_STREAMS, nw, f_cols), np.float32)
    for p in range(N_PAIRS):
        s, fb = pos8[p]
        for sl in range(2):
            y[2 * p + sl] = yv[:, s, sl, fb, :]
    return np.ascontiguousarray(y).reshape(N_STREAMS * l32, 1)


def build_program_tiles(n_core=N_CORE, f_cols=512, repeat=1,
                        num_devices=N_CORES, depth=2, xbufs=None, hbufs=None,
                        ybufs=None, dense_lout=False):
    """Tile-position wave kernel.  One wave = 16 concurrent 32x32 MMs
    (f_cols columns each) into a [128, 4*f_cols] PSUM tile (2 banks at
    f_cols=256), one ACT pass per wave-layer.  `depth` wave-chains are
    interleaved so ACT stays busy while the other chain's MMs run."""
    l32 = n_core // N_STREAMS
    nw = l32 // f_cols
    F = f_cols
    pos = _pair_positions()

    nc = bacc.Bacc("TRN2", target_bir_lowering=False, debug=False,
                   num_devices=num_devices)
    xt_ap = nc.dram_tensor("xt", [24, l32 * 4], F32, kind="ExternalInput").ap()
    w0_ap = nc.dram_tensor("w0t32", [128, 32], F32, kind="ExternalInput").ap()
    wh_ap = nc.dram_tensor("wh32", [128, 32 * N_HIDDEN], F32,
                           kind="ExternalInput").ap()
    wo_ap = nc.dram_tensor("wo32", [128, 32], F32, kind="ExternalInput").ap()
    wod_ap = nc.dram_tensor("wod", [128, 8], F32, kind="ExternalInput").ap()
    bias_ap = nc.dram_tensor("bias32", [128, 10], F32,
                             kind="ExternalInput").ap()
    # only rows 32c+sl of the sigmoid output carry data; ship them packed
    if dense_lout:
        y_ap = nc.dram_tensor("y_raw", [8 * nw, 4 * f_cols], F32,
                              kind="ExternalOutput").ap()
    else:
        y_ap = nc.dram_tensor("y_raw", [8, l32 * 4], F32,
                              kind="ExternalOutput").ap()

    Tanh = mybir.ActivationFunctionType.Tanh
    Sigmoid = mybir.ActivationFunctionType.Sigmoid

    with tile.TileContext(nc) as tc, ExitStack() as ctx:
        consts = ctx.enter_context(tc.tile_pool(name="consts", bufs=1))
        xpool = ctx.enter_context(tc.tile_pool(
            name="xpool", bufs=xbufs or depth + 1))
        hpool = ctx.enter_context(tc.tile_pool(
            name="hpool", bufs=hbufs or 2 * depth + 1))
        ypool = ctx.enter_context(tc.tile_pool(
            name="ypool", bufs=ybufs or depth + 1))
        # one PSUM slot is [128, 4F] with free-block i = one full bank at
        # F=512, so concurrent tile drains never share a (bank, partition)
        zbufs = max(2, 8 // max(1, (4 * f_cols * 4) // 2048))
        zpool = ctx.enter_context(tc.tile_pool(name="zpool", bufs=zbufs,
                                               space="PSUM"))

        w0_sb = consts.tile([128, 32], F32)
        nc.sync.dma_start(w0_sb[:], w0_ap[:])
        wh_sb = consts.tile([128, 32 * N_HIDDEN], F32)
        nc.sync.dma_start(wh_sb[:], wh_ap[:])
        wo_sb = consts.tile([128, 32], F32)
        nc.sync.dma_start(wo_sb[:], wo_ap[:])
        wod_sb = consts.tile([128, 8], F32)
        nc.sync.dma_start(wod_sb[:], wod_ap[:])
        bias_sb = consts.tile([128, 10], F32)
        nc.sync.dma_start(bias_sb[:], bias_ap[:])

        for _rep in range(repeat):
            if dense_lout:
                ystage = ypool.tile([8 * nw, 4 * f_cols], F32, tag="ystage",
                                    bufs=1)
            for w0i in range(0, nw, depth):
                ws = range(w0i, min(w0i + depth, nw))
                xss, hs, zs = {}, {}, {}
                for wv in ws:
                    xs = xpool.tile([128, 4 * F], F32, tag="xs")
                    for b in range(4):
                        nc.sync.dma_start(
                            xs[32 * b:32 * b + 6, :],
                            xt_ap[6 * b:6 * b + 6,
                                  wv * 4 * F:(wv + 1) * 4 * F])
                    xss[wv] = xs

                # ---- layer 0 wave: pair (a,b): tile (b, a), rhs strip b
                # fb a, out (strip a, fb b)
                for wv in ws:
                    z = zpool.tile([128, 4 * F], F32, tag="z")
                    zs[wv] = z
                    for p in range(N_PAIRS):
                        a, b = p // 4, p % 4
                        nc.tensor.matmul(
                            z[32 * a:32 * (a + 1), b * F:(b + 1) * F],
                            lhsT=w0_sb[32 * b:32 * b + 6, :],
                            rhs=xss[wv][32 * b:32 * b + 6,
                                        a * F:(a + 1) * F],
                            start=True, stop=True,
                            tile_position=(32 * b, 32 * a),
                            skip_group_check=True,
                        )
                for wv in ws:
                    h = hpool.tile([128, 4 * F], F32, tag="h")
                    nc.scalar.activation(h[:], zs[wv][:], Tanh,
                                         bias=bias_sb[:, 0:1])
                    hs[wv] = h

                # ---- hidden waves
                for l in range(N_HIDDEN):
                    cur, nxt = pos[l], pos[l + 1]
                    for wv in ws:
                        z = zpool.tile([128, 4 * F], F32, tag="z")
                        zs[wv] = z
                        for p in range(N_PAIRS):
                            s, fb = cur[p]
                            s2 = nxt[p][0]
                            nc.tensor.matmul(
                                z[32 * s2:32 * (s2 + 1), s * F:(s + 1) * F],
                                lhsT=wh_sb[32 * s:32 * (s + 1),
                                           32 * l:32 * (l + 1)],
                                rhs=hs[wv][32 * s:32 * (s + 1),
                                           fb * F:(fb + 1) * F],
                                start=True, stop=True,
                                tile_position=(32 * s, 32 * s2),
                                skip_group_check=True,
                            )
                    for wv in ws:
                        h2 = hpool.tile([128, 4 * F], F32, tag="h")
                        nc.scalar.activation(h2[:], zs[wv][:], Tanh,
                                             bias=bias_sb[:, l + 1:l + 2])
                        hs[wv] = h2

                # ---- output wave
                cur, nxt = pos[N_HIDDEN], pos[N_HIDDEN + 1]
                if dense_lout:
                    # one full-array MM per f_cols chunk -> dense [8, 4F]
                    for wv in ws:
                        zo = zpool.tile([128, 4 * F], F32, tag="z")
                        zs[wv] = zo
                        for c in range(4):
                            nc.tensor.matmul(
                                zo[0:8, c * F:(c + 1) * F],
                                lhsT=wod_sb[:],
                                rhs=hs[wv][:, c * F:(c + 1) * F],
                                start=True, stop=True,
                            )
                    for wv in ws:
                        tmp = ypool.tile([8, 4 * F], F32, tag="ys")
                        nc.vector.tensor_copy(tmp[:], zs[wv][0:8, :])
                        nc.sync.dma_start(
                            ystage[8 * wv:8 * wv + 8, :], tmp[:])
                else:
                    for wv in ws:
                        zo = zpool.tile([128, 4 * F], F32, tag="z")
                        zs[wv] = zo
                        for p in range(N_PAIRS):
                            s, fb = cur[p]
                            s2 = nxt[p][0]
                            nc.tensor.matmul(
                                zo[32 * s2:32 * (s2 + 1), s * F:(s + 1) * F],
                                lhsT=wo_sb[32 * s:32 * (s + 1), :],
                                rhs=hs[wv][32 * s:32 * (s + 1),
                                           fb * F:(fb + 1) * F],
                                start=True, stop=True,
                                tile_position=(32 * s, 32 * s2),
                                skip_group_check=True,
                            )
                    for wv in ws:
                        ys = ypool.tile([128, 4 * F], F32, tag="ys")
                        nc.scalar.activation(ys[:], zs[wv][:], Sigmoid,
                                             bias=bias_sb[:, 9:10])
                        for c in range(4):
                            nc.sync.dma_start(
                                y_ap[2 * c:2 * c + 2,
                                     wv * 4 * F:(wv + 1) * 4 * F],
                                ys[32 * c:32 * c + 2, :])

            if dense_lout:
                ysig = ypool.tile([8 * nw, 4 * f_cols], F32, tag="ysig",
                                  bufs=1)
                nc.scalar.activation(ysig[:], ystage[:], Sigmoid,
                                     bias=bias_sb[0:8 * nw, 9:10])
                nc.sync.dma_start(y_ap[:], ysig[:])

    nc.compile()
    return nc


# ---------------------------------------------------------------------------
# v2 "pairsig": tiles kernel with pair-dense output + DMA queue separation.
#
# Changes vs build_program_tiles (measured 396 us; ACT busy only ~283 us):
#  1. All DMAs shared the in-order SP queue, so each pair's x prefetch queued
#     behind sigmoid-gated y stores => ~13 us ACT bubble per pair.  Now x
#     loads alternate SP/DVE queues and y stores ride the GPSIMD queue.
#  2. Output layer: the two waves of a pair accumulate into one [16, 4F]
#     PSUM tile via two full-array float32r matmuls with zero-padded
#     stationaries (wodA | wodB), so sigmoid runs once per pair (8 ACT
#     instrs instead of 16).  f32r's 2^-13 noise is fine on the final layer
#     (no amplification through later layers).
#  3. Pair p's L0 matmuls are emitted BEFORE pair p-1's output matmuls so
#     the PE always has the next z ready when ACT frees a PSUM slot.
# ACT floor: (144 tanh + 8 sigmoid) x ~1.78 us ~= 270 us/core.
# ---------------------------------------------------------------------------


def format_inputs_tiles2(W0, b0, Wh, bh, Wo, bo):
    base = format_inputs_tiles(W0, b0, Wh, bh, Wo, bo)
    Wo = np.asarray(Wo, np.float32)
    # Pair-dense output stationaries [128, 16]: chain A -> out rows 0..7,
    # chain B -> rows 8..15; row index r = 2s+sl contracts h rows 32s+16sl.
    wodA = np.zeros((128, 16), np.float32)
    wodB = np.zeros((128, 16), np.float32)
    for s in range(4):
        for sl in range(2):
            r0 = 32 * s + 16 * sl
            wodA[r0:r0 + W, 2 * s + sl] = Wo[0, :]
            wodB[r0:r0 + W, 8 + 2 * s + sl] = Wo[0, :]
    base["wodA"] = round_f32r(wodA)
    base["wodB"] = round_f32r(wodB)
    # Column-dense variant: one [128, 64] stationary per (chunk, chain),
    # packed into [128, 512].  Block i = 2c+ch steers chunk c of chain ch to
    # out rows 16c+8ch+2s+sl, col n (all 8 MMs accumulate into [64, 512]).
    wod64 = np.zeros((128, 8 * 64), np.float32)
    for c in range(4):
        for ch in range(2):
            i = 2 * c + ch
            for s in range(4):
                for sl in range(2):
                    r0 = 32 * s + 16 * sl
                    m = 16 * c + 8 * ch + 2 * s + sl
                    wod64[r0:r0 + W, 64 * i + m] = Wo[0, :]
    base["wod64"] = round_f32r(wod64)
    return base


def host_unpack_y_pairsig(y_raw, f_cols=512):
    """y_raw [128, 4F]: row 16p + 8*ch + 2s+sl, col fb*F+n holds the output
    of pair pp (pos8[pp] = (s, fb)), slot sl, wave w = 2p+ch, point n."""
    F = f_cols
    npair = y_raw.shape[0] // 16
    pos8 = _pair_positions()[N_HIDDEN]
    yv = y_raw.reshape(npair, 2, 4, 2, 4, F)    # [p, ch, s, sl, fb, n]
    nwav = 2 * npair
    l32 = nwav * F
    y = np.empty((N_STREAMS, nwav, F), np.float32)
    for pp in range(N_PAIRS):
        s, fb = pos8[pp]
        for sl in range(2):
            y[2 * pp + sl] = yv[:, :, s, sl, fb, :].reshape(nwav, F)
    return np.ascontiguousarray(y).reshape(N_STREAMS * l32, 1)


def host_unpack_y_pairsig64(y_raw, f_cols=512):
    """Column-dense y_raw [64*npair, F]: row 64p + 16fb + 8ch + 2s+sl,
    col n holds pair pp (pos8[pp] = (s, fb)), slot sl, wave 2p+ch."""
    F = f_cols
    npair = y_raw.shape[0] // 64
    pos8 = _pair_positions()[N_HIDDEN]
    yv = y_raw.reshape(npair, 4, 2, 4, 2, F)    # [p, fb, ch, s, sl, n]
    nwav = 2 * npair
    l32 = nwav * F
    y = np.empty((N_STREAMS, nwav, F), np.float32)
    for pp in range(N_PAIRS):
        s, fb = pos8[pp]
        for sl in range(2):
            y[2 * pp + sl] = yv[:, fb, :, s, sl, :].reshape(nwav, F)
    return np.ascontiguousarray(y).reshape(N_STREAMS * l32, 1)


def build_program_tiles2(n_core=N_CORE, f_cols=512, repeat=1,
                         num_devices=N_CORES, xbufs=5, hbufs=7, ybufs=3,
                         out_f32r=True):
    l32 = n_core // N_STREAMS
    nw = l32 // f_cols
    F = f_cols
    assert nw % 2 == 0
    pos = _pair_positions()

    npair = nw // 2
    nc = bacc.Bacc("TRN2", target_bir_lowering=False, debug=False,
                   num_devices=num_devices)
    xt_ap = nc.dram_tensor("xt", [24, l32 * 4], F32, kind="ExternalInput").ap()
    w0_ap = nc.dram_tensor("w0t32", [128, 32], F32, kind="ExternalInput").ap()
    wh_ap = nc.dram_tensor("wh32", [128, 32 * N_HIDDEN], F32,
                           kind="ExternalInput").ap()
    wod64_ap = nc.dram_tensor("wod64", [128, 8 * 64], F32R,
                              kind="ExternalInput").ap()
    bias_ap = nc.dram_tensor("bias32", [128, 10], F32,
                             kind="ExternalInput").ap()
    y_ap = nc.dram_tensor("y_raw", [64 * npair, F], F32,
                          kind="ExternalOutput").ap()

    Tanh = mybir.ActivationFunctionType.Tanh
    Sigmoid = mybir.ActivationFunctionType.Sigmoid
    xq = [nc.sync, nc.gpsimd]       # x strips alternate the two DGE queues
    yq = nc.gpsimd                  # y stores: SWDGE, behind that pair's x

    with tile.TileContext(nc) as tc, ExitStack() as ctx:
        consts = ctx.enter_context(tc.tile_pool(name="consts", bufs=1))
        xpool = ctx.enter_context(tc.tile_pool(name="xpool", bufs=xbufs))
        hpool = ctx.enter_context(tc.tile_pool(name="hpool", bufs=hbufs))
        ypool = ctx.enter_context(tc.tile_pool(name="ypool", bufs=ybufs))
        zpool = ctx.enter_context(tc.tile_pool(name="zpool", bufs=2,
                                               space="PSUM"))

        w0_sb = consts.tile([128, 32], F32)
        nc.sync.dma_start(w0_sb[:], w0_ap[:])
        wh_sb = consts.tile([128, 32 * N_HIDDEN], F32)
        nc.sync.dma_start(wh_sb[:], wh_ap[:])
        wod64_sb = consts.tile([128, 8 * 64], F32R)
        nc.sync.dma_start(wod64_sb[:], wod64_ap[:])
        bias_sb = consts.tile([128, 10], F32)
        nc.sync.dma_start(bias_sb[:], bias_ap[:])

        def load_x(wv):
            xs = xpool.tile([128, 4 * F], F32, tag="xs")
            for b in range(4):
                xq[b % 2].dma_start(
                    xs[32 * b:32 * b + 6, :],
                    xt_ap[6 * b:6 * b + 6, wv * 4 * F:(wv + 1) * 4 * F])
            return xs

        def l0_wave(xs):
            z = zpool.tile([128, 4 * F], F32, tag="z")
            for p in range(N_PAIRS):
                a, b = p // 4, p % 4
                nc.tensor.matmul(
                    z[32 * a:32 * (a + 1), b * F:(b + 1) * F],
                    lhsT=w0_sb[32 * b:32 * b + 6, :],
                    rhs=xs[32 * b:32 * b + 6, a * F:(a + 1) * F],
                    start=True, stop=True,
                    tile_position=(32 * b, 32 * a),
                    skip_group_check=True,
                )
            return z

        def hidden_wave(h, l):
            cur, nxt = pos[l], pos[l + 1]
            z = zpool.tile([128, 4 * F], F32, tag="z")
            for p in range(N_PAIRS):
                s, fb = cur[p]
                s2 = nxt[p][0]
                nc.tensor.matmul(
                    z[32 * s2:32 * (s2 + 1), s * F:(s + 1) * F],
                    lhsT=wh_sb[32 * s:32 * (s + 1), 32 * l:32 * (l + 1)],
                    rhs=h[32 * s:32 * (s + 1), fb * F:(fb + 1) * F],
                    start=True, stop=True,
                    tile_position=(32 * s, 32 * s2),
                    skip_group_check=True,
                )
            return z

        def act(z, col, tag="h", dtype=F32):
            h = hpool.tile([128, 4 * F], dtype, tag=tag)
            nc.scalar.activation(h[:], z[:], Tanh,
                                 bias=bias_sb[:, col:col + 1])
            return h

        def out_mms(hA, hB):
            """Column-dense output: 8 accumulating f32r MMs steer chunk c of
            chain ch to rows 16c+8ch+..., giving [64, F] — so the sigmoid's
            free size is F, not 4F.  hA/hB are F32R (rounded on ACT write),
            so the MMs run at 1 cyc/row and stay off the critical path."""
            zo = zpool.tile([64, F], F32, tag="z")
            for c in range(4):
                for ch, h in enumerate((hA, hB)):
                    i = 2 * c + ch
                    nc.tensor.matmul(
                        zo[0:64, :],
                        lhsT=wod64_sb[:, 64 * i:64 * (i + 1)],
                        rhs=h[:, c * F:(c + 1) * F],
                        start=(i == 0), stop=(i == 7),
                    )
            return zo

        def sig_out(zo, p, last=False):
            ys = ypool.tile([64, F], F32, tag="ys")
            nc.scalar.activation(ys[:], zo[:], Sigmoid,
                                 bias=bias_sb[0:64, 9:10])
            if last:    # split the tail store across both queues
                yq.dma_start(y_ap[64 * p:64 * p + 32, :], ys[0:32, :])
                nc.sync.dma_start(y_ap[64 * p + 32:64 * p + 64, :],
                                  ys[32:64, :])
            else:
                yq.dma_start(y_ap[64 * p:64 * p + 64, :], ys[:])

        for _rep in range(repeat):
            prev = None     # (zo emitted?, hA8, hB8, pair_idx)
            for p in range(nw // 2):
                wA, wB = 2 * p, 2 * p + 1
                xsA = load_x(wA)
                xsB = load_x(wB)
                zA = l0_wave(xsA)
                if prev is not None:
                    zo = out_mms(prev[0], prev[1])
                hA = act(zA, 0)
                if prev is not None:
                    sig_out(zo, prev[2])
                zB = l0_wave(xsB)
                hB = act(zB, 0)
                for l in range(N_HIDDEN):
                    # the last hidden tanh feeds only the f32r output MMs;
                    # write it pre-rounded to the f32r grid
                    dt = F32R if (out_f32r and l == N_HIDDEN - 1) else F32
                    zA = hidden_wave(hA, l)
                    hA = act(zA, l + 1, dtype=dt)
                    zB = hidden_wave(hB, l)
                    hB = act(zB, l + 1, dtype=dt)
                prev = (hA, hB, p)
            zo = out_mms(prev[0], prev[1])
            sig_out(zo, prev[2], last=True)

    nc.compile()
    return nc


_RUNNER_CACHE = {}
L0_MODE = "host_t"
KERNEL_MODE = "tiles2"  # "tiles2" | "tiles" | "block"


def host_transpose_x(x_core):
    """[n_core, 3] -> [24, n_core/S]: partition 8f+j = feature f of stream j."""
    L = x_core.shape[0] // S
    return np.ascontiguousarray(
        x_core.reshape(S, L, 3).transpose(2, 0, 1).reshape(24, L))


def make_in_maps(x, consts, l0_mode=None):
    """Per-core input maps from the full x [N_FULL, 3] + formatted weights."""
    l0_mode = l0_mode or L0_MODE
    in_maps = []
    for c in range(N_CORES):
        xc = x[c * N_CORE:(c + 1) * N_CORE]
        if l0_mode == "host_t":
            m = {"xt": host_transpose_x(xc)}
        else:
            m = {"x": np.ascontiguousarray(xc)}
        m.update(consts)
        in_maps.append(m)
    return in_maps


def make_runner(nc, n_cores=N_CORES):
    """Build a reusable jitted PJRT runner for the SPMD program (mirrors
    bass2jax.run_bass_via_pjrt's multi-core path, minus output donation, so
    the NEFF compile is paid once and later calls are just execution)."""
    import jax
    from jax.sharding import Mesh, PartitionSpec, NamedSharding
    from jax.experimental.shard_map import shard_map
    from concourse import bass2jax

    bass2jax.install_neuronx_cc_hook()
    partition_name = (nc.partition_id_tensor.name
                      if nc.partition_id_tensor else None)
    in_names, out_names, out_avals = [], [], []
    for alloc in nc.m.functions[0].allocations:
        if not isinstance(alloc, mybir.MemoryLocationSet):
            continue
        name = alloc.memorylocations[0].name
        if alloc.kind == "ExternalInput":
            if name != partition_name:
                in_names.append(name)
        elif alloc.kind == "ExternalOutput":
            out_names.append(name)
            out_avals.append(jax.core.ShapedArray(
                tuple(alloc.tensor_shape), mybir.dt.np(alloc.dtype)))
    n_params = len(in_names)
    all_in_names = list(in_names) + list(out_names)
    if partition_name is not None:
        all_in_names.append(partition_name)

    def _body(*args):
        operands = list(args)
        if partition_name is not None:
            operands.append(bass2jax.partition_id_tensor())
        outs = bass2jax._bass_exec_p.bind(
            *operands,
            out_avals=tuple(out_avals),
            in_names=tuple(all_in_names),
            out_names=tuple(out_names),
            lowering_input_output_aliases=(),
            sim_require_finite=True,
            sim_require_nnan=True,
            nc=nc,
        )
        return tuple(outs)

    devices = jax.devices()[:n_cores]
    mesh = Mesh(np.asarray(devices), ("core",))
    n_outs = len(out_names)
    in_specs = (PartitionSpec("core"),) * (n_params + n_outs)
    out_specs = (PartitionSpec("core"),) * n_outs
    fn = jax.jit(shard_map(_body, mesh=mesh, in_specs=in_specs,
                           out_specs=out_specs, check_rep=False),
                 keep_unused=True)
    sharding = NamedSharding(mesh, PartitionSpec("core"))

    def prepare(in_maps):
        concat_in = [
            np.concatenate([np.asarray(in_maps[c][n])
                            for c in range(n_cores)], axis=0)
            for n in in_names
        ]
        concat_zero = [
            np.zeros((n_cores * a.shape[0],) + tuple(a.shape[1:]), a.dtype)
            for a in out_avals
        ]
        return [jax.device_put(a, sharding) for a in concat_in + concat_zero]

    return fn, prepare, out_names


def _get_runner(key=(N_CORE, 1024, 512, 1, L0_MODE)):
    if key not in _RUNNER_CACHE:
        n_core, g_cols, f_cols, repeat, l0_mode = key
        nc = build_program(n_core, g_cols, f_cols, repeat, l0_mode=l0_mode)
        _RUNNER_CACHE[key] = make_runner(nc)
    return _RUNNER_CACHE[key]


def _get_runner_tiles(key=(N_CORE, 512, 1)):
    if key not in _RUNNER_CACHE:
        n_core, f_cols, repeat = key
        nc = build_program_tiles(n_core, f_cols, repeat)
        _RUNNER_CACHE[key] = make_runner(nc)
    return _RUNNER_CACHE[key]


def _get_runner_tiles2(key=(N_CORE, 512, 1)):
    if ("t2",) + key not in _RUNNER_CACHE:
        n_core, f_cols, repeat = key
        nc = build_program_tiles2(n_core, f_cols, repeat)
        _RUNNER_CACHE[("t2",) + key] = make_runner(nc)
    return _RUNNER_CACHE[("t2",) + key]


def kernel(x, W0, b0, Wh, bh, Wo, bo):
    import jax
    x = np.asarray(x, np.float32)
    assert x.shape == (N_FULL, 3), x.shape
    if KERNEL_MODE == "tiles2":
        consts = format_inputs_tiles2(W0, b0, Wh, bh, Wo, bo)
        fn, prepare, out_names = _get_runner_tiles2()
        in_maps = []
        for c in range(N_CORES):
            m = {"xt": host_pack_x_tiles(x[c * N_CORE:(c + 1) * N_CORE])}
            m.update({k: consts[k] for k in
                      ("w0t32", "wh32", "wod64", "bias32")})
            in_maps.append(m)
        args = prepare(in_maps)
        outs = fn(*args)
        jax.block_until_ready(outs)
        y_raw = np.asarray(outs[out_names.index("y_raw")])
        y_raw = y_raw.reshape(N_CORES, 512, -1)
        y = np.concatenate(
            [host_unpack_y_pairsig64(y_raw[c]) for c in range(N_CORES)],
            axis=0)
        return np.ascontiguousarray(y.astype(np.float32))
    if KERNEL_MODE == "tiles":
        consts = format_inputs_tiles(W0, b0, Wh, bh, Wo, bo)
        fn, prepare, out_names = _get_runner_tiles()
        in_maps = []
        for c in range(N_CORES):
            m = {"xt": host_pack_x_tiles(x[c * N_CORE:(c + 1) * N_CORE])}
            m.update(consts)
            in_maps.append(m)
        args = prepare(in_maps)
        outs = fn(*args)
        jax.block_until_ready(outs)
        y_raw = np.asarray(outs[out_names.index("y_raw")])
        y_raw = y_raw.reshape(N_CORES, 8, L32 * 4)
        y = np.concatenate(
            [host_unpack_y_tiles(y_raw[c]) for c in range(N_CORES)], axis=0)
        return np.ascontiguousarray(y.astype(np.float32))
    consts = format_inputs(W0, b0, Wh, bh, Wo, bo)
    fn, prepare, out_names = _get_runner()
    args = prepare(make_in_maps(x, consts))
    outs = fn(*args)
    jax.block_until_ready(outs)
    y = np.asarray(outs[out_names.index("y")])
    return np.ascontiguousarray(y.reshape(N_FULL, 1).astype(np.float32))



# revision 18
# speedup vs baseline: 1.0803x; 1.0448x over previous
"""CPPN dense-MLP Trainium2 kernel.

Network (per point): 3 -> 16 (tanh) -> 8 x [16 -> 16 (tanh)] -> 1 (sigmoid).
2,097,152 points, pure data parallel across 8 NeuronCores.

Per-core layout: the core's 262,144 points are split into S=8 streams of
32,768 points.  Activations live in SBUF/PSUM "block layout": partition
16*j + m holds feature m of stream j, free dim indexes points within the
stream.  Every layer is then a single 128x128 block-diagonal stationary
matmul on the tensor engine (8 independent 16x16 matmuls per cycle).

Layer 0 (K=3) needs x with features on partitions; the host pre-transposes
each core's shard to [24, 32768] (partition 8f+j = feature f of stream j;
a cheap numpy reshape, part of sharding), so layer 0 is a single K=24
matmul with fully contiguous DMA loads.  (Alternative modes kept for A/B:
"three_mm" = stride-3 rhs APs over natural-layout x, ~+110us PE;
"strided" = DMA-side transpose, ~+600us of 4-byte-element descriptors.)

Activations (tanh / sigmoid + bias) run on the scalar engine directly from
PSUM into SBUF; tanh and sigmoid share one ACT table set so there are no
table reloads.

Matmuls are full float32 (4 cycles/column): this network doubles any
injected error every layer (~x250 over the 9-layer chain), so float32r's
~2^-13 per-product noise lands at ~0.3 absolute output error while fp32
stays at ~3e-5.  fp32 is mandatory for the gate, making the kernel
PE-bound.
"""

import numpy as np
import ml_dtypes
from contextlib import ExitStack


def round_f32r(a):
    """Round fp32 to the float32r grid (value representable as bf16 + bf16)."""
    a = np.asarray(a, np.float32)
    hi = a.astype(ml_dtypes.bfloat16).astype(np.float32)
    lo = (a - hi).astype(ml_dtypes.bfloat16).astype(np.float32)
    return hi + lo

import concourse.bass as bass
import concourse.tile as tile
from concourse import bacc, mybir
from concourse.bass_utils import run_bass_kernel_spmd

F32 = mybir.dt.float32
F32R = mybir.dt.float32r

N_FULL = 2097152
N_CORES = 8
N_CORE = N_FULL // N_CORES  # 262144 points per core
S = 8                       # streams per core
W = 16                      # hidden width
N_HIDDEN = 8


def format_inputs(W0, b0, Wh, bh, Wo, bo):
    """Build the block-diagonal stationary matrices + bias table (numpy)."""
    W0 = np.asarray(W0, np.float32)
    b0 = np.asarray(b0, np.float32)
    Wh = np.asarray(Wh, np.float32)
    bh = np.asarray(bh, np.float32)
    Wo = np.asarray(Wo, np.float32)
    bo = np.asarray(bo, np.float32)

    # Layer 0: three [S, 128] stationaries (one per input feature), packed
    # side by side into [S, 3*128].  stationary_f[j, 16j+m] = W0[m, f].
    w0f = np.zeros((S, 3 * 128), np.float32)
    for f in range(3):
        for j in range(S):
            w0f[j, f * 128 + 16 * j:f * 128 + 16 * j + W] = W0[:, f]

    # Hidden layers: [128, 8*128]; slice l is blockdiag(Wh[l].T x8):
    # stat[16j+fi, 16j+m] = Wh[l][m, fi].
    wh = np.zeros((128, N_HIDDEN * 128), np.float32)
    for l in range(N_HIDDEN):
        for j in range(S):
            r = 16 * j
            wh[r:r + W, l * 128 + r:l * 128 + r + W] = Wh[l].T

    # Output layer: [128, S]: stat[16j+fi, j] = Wo[0, fi].
    wo = np.zeros((128, S), np.float32)
    for j in range(S):
        wo[16 * j:16 * j + W, j] = Wo[0, :]

    # Bias table [128, 10]: col 0 = b0 block, cols 1..8 = bh blocks,
    # col 9 rows 0..7 = bo.
    bias = np.zeros((128, 10), np.float32)
    for j in range(S):
        bias[16 * j:16 * j + W, 0] = b0
        for l in range(N_HIDDEN):
            bias[16 * j:16 * j + W, 1 + l] = bh[l]
    bias[0:S, 9] = bo[0]

    # Layer 0 alternative: single [24, 128] stationary for K=24 matmul
    # over a DMA-transposed x (f-major partitions): w0t[8f+j, 16j+m] = W0[m, f].
    w0t = np.zeros((24, 128), np.float32)
    for j in range(S):
        for f in range(3):
            w0t[8 * f + j, 16 * j:16 * j + W] = W0[:, f]

    return {"w0f": w0f, "w0t": w0t, "wh": wh, "wo": wo, "bias": bias}


def build_program(n_core=N_CORE, g_cols=1024, f_cols=512, repeat=1,
                  num_devices=N_CORES, depth=2, xbufs=None, hbufs=None,
                  ybufs=None, zbufs=None, l0_mode="three_mm"):
    """Build + compile the per-core Bass program (SPMD: same on all cores).

    n_core: points per core.  g_cols: free-dim columns per group (pipeline
    granularity; one ACT instruction per layer per group).  f_cols: free-dim
    columns per matmul (<= 512, one PSUM bank).  repeat: run the whole
    kernel body this many times (for wall-clock timing; output idempotent).
    """
    stream_len = n_core // S
    ng = stream_len // g_cols
    assert stream_len % g_cols == 0 and g_cols % f_cols == 0
    cpg = g_cols // f_cols  # matmul chunks per group

    nc = bacc.Bacc("TRN2", target_bir_lowering=False, debug=False,
                   num_devices=num_devices)
    if l0_mode == "host_t":
        xt_ap = nc.dram_tensor("xt", [24, n_core // S], F32,
                               kind="ExternalInput").ap()
        x_ap = None
    else:
        x_ap = nc.dram_tensor("x", [n_core, 3], F32,
                              kind="ExternalInput").ap()
    w0f_ap = nc.dram_tensor("w0f", [S, 3 * 128], F32, kind="ExternalInput").ap()
    w0t_ap = nc.dram_tensor("w0t", [24, 128], F32, kind="ExternalInput").ap()
    wh_ap = nc.dram_tensor("wh", [128, N_HIDDEN * 128], F32,
                           kind="ExternalInput").ap()
    wo_ap = nc.dram_tensor("wo", [128, S], F32, kind="ExternalInput").ap()
    bias_ap = nc.dram_tensor("bias", [128, 10], F32, kind="ExternalInput").ap()
    y_ap = nc.dram_tensor("y", [n_core, 1], F32, kind="ExternalOutput").ap()

    # DRAM views: stream-major.  x[(j n) f] -> [S, stream_len*3] so each
    # partition's group slice is one contiguous run.
    if x_ap is not None:
        xr = x_ap.rearrange("(j n) f -> j (n f)", j=S)
        # transposed view for strided-DMA layer 0: [S, 3, stream_len]
        xt_dram = x_ap.rearrange("(j n) f -> j f n", j=S)
    yr = y_ap.rearrange("(j n) o -> j (n o)", j=S)

    Tanh = mybir.ActivationFunctionType.Tanh
    Sigmoid = mybir.ActivationFunctionType.Sigmoid

    with tile.TileContext(nc) as tc, ExitStack() as ctx:
        if zbufs is None:
            # slots are bank (2KB) granular; fill the 8 banks
            banks_per_slot = max(1, (g_cols * 4) // 2048)
            zbufs = max(2, 8 // banks_per_slot)
        if xbufs is None:
            xbufs = depth + 1
        if hbufs is None:
            hbufs = 2 * depth + 1
        if ybufs is None:
            ybufs = depth + 1
        consts = ctx.enter_context(tc.tile_pool(name="consts", bufs=1))
        xpool = ctx.enter_context(tc.tile_pool(name="xpool", bufs=xbufs))
        hpool = ctx.enter_context(tc.tile_pool(name="hpool", bufs=hbufs))
        ypool = ctx.enter_context(tc.tile_pool(name="ypool", bufs=ybufs))
        zpool = ctx.enter_context(
            tc.tile_pool(name="zpool", bufs=zbufs, space="PSUM"))

        w0f_sb = consts.tile([S, 3 * 128], F32)
        nc.sync.dma_start(w0f_sb[:], w0f_ap[:])
        w0t_sb = consts.tile([24, 128], F32)
        nc.sync.dma_start(w0t_sb[:], w0t_ap[:])
        wh_sb = consts.tile([128, N_HIDDEN * 128], F32)
        nc.sync.dma_start(wh_sb[:], wh_ap[:])
        wo_sb = consts.tile([128, S], F32)
        nc.sync.dma_start(wo_sb[:], wo_ap[:])
        bias_sb = consts.tile([128, 10], F32)
        nc.sync.dma_start(bias_sb[:], bias_ap[:])

        # Interleave `depth` groups at each layer step: within a group the
        # PE's layer l+1 strictly follows ACT of layer l, so a single group
        # serializes PE<->ACT.  Emitting layer l for D groups back-to-back
        # gives the PE work while ACT drains the other groups' PSUM tiles.
        for _rep in range(repeat):
            for g0 in range(0, ng, depth):
                gs = range(g0, min(g0 + depth, ng))
                xss, hs, zs = {}, {}, {}
                for g in gs:
                    if l0_mode == "three_mm":
                        # ---- load x chunk [S, 3*g_cols], contiguous/stream
                        xs = xpool.tile([S, 3 * g_cols], F32, tag="xs")
                        nc.sync.dma_start(
                            xs[:], xr[:, g * 3 * g_cols:(g + 1) * 3 * g_cols])
                        # stride-3 feature view: [S, g_cols, 3]
                        xss[g] = xs.rearrange("p (n f) -> p n f", f=3)
                    elif l0_mode == "host_t":
                        # ---- x pre-transposed on host: contiguous [24, g]
                        xs = xpool.tile([24, g_cols], F32, tag="xs")
                        nc.sync.dma_start(
                            xs[:], xt_ap[:, g * g_cols:(g + 1) * g_cols])
                        xss[g] = xs
                    else:
                        # ---- strided-DMA transpose load: [24, g_cols]
                        xs = xpool.tile([24, g_cols], F32, tag="xs")
                        for f in range(3):
                            nc.sync.dma_start(
                                xs[8 * f:8 * (f + 1), :],
                                xt_dram[:, f, g * g_cols:(g + 1) * g_cols])
                        xss[g] = xs

                # ---- layer 0
                for g in gs:
                    z = zpool.tile([128, g_cols], F32, tag="z")
                    zs[g] = z
                    for c in range(cpg):
                        if l0_mode == "three_mm":
                            for f in range(3):
                                nc.tensor.matmul(
                                    z[:, c * f_cols:(c + 1) * f_cols],
                                    lhsT=w0f_sb[:, f * 128:(f + 1) * 128],
                                    rhs=xss[g][:, c * f_cols:(c + 1) * f_cols,
                                               f],
                                    start=(f == 0), stop=(f == 2),
                                )
                        else:  # host_t / strided: single K=24 matmul
                            nc.tensor.matmul(
                                z[:, c * f_cols:(c + 1) * f_cols],
                                lhsT=w0t_sb[:],
                                rhs=xss[g][:, c * f_cols:(c + 1) * f_cols],
                                start=True, stop=True,
                            )
                for g in gs:
                    h = hpool.tile([128, g_cols], F32, tag="h")
                    nc.scalar.activation(h[:], zs[g][:], Tanh,
                                         bias=bias_sb[:, 0:1])
                    hs[g] = h

                # ---- hidden layers
                for l in range(N_HIDDEN):
                    for g in gs:
                        z = zpool.tile([128, g_cols], F32, tag="z")
                        zs[g] = z
                        for c in range(cpg):
                            nc.tensor.matmul(
                                z[:, c * f_cols:(c + 1) * f_cols],
                                lhsT=wh_sb[:, l * 128:(l + 1) * 128],
                                rhs=hs[g][:, c * f_cols:(c + 1) * f_cols],
                                start=True, stop=True,
                            )
                    for g in gs:
                        h2 = hpool.tile([128, g_cols], F32, tag="h")
                        nc.scalar.activation(h2[:], zs[g][:], Tanh,
                                             bias=bias_sb[:, l + 1:l + 2])
                        hs[g] = h2

                # ---- output layer -> [S, g_cols]
                for g in gs:
                    zo = zpool.tile([S, g_cols], F32, tag="z")
                    zs[g] = zo
                    for c in range(cpg):
                        nc.tensor.matmul(
                            zo[:, c * f_cols:(c + 1) * f_cols],
                            lhsT=wo_sb[:],
                            rhs=hs[g][:, c * f_cols:(c + 1) * f_cols],
                            start=True, stop=True,
                        )
                for g in gs:
                    ys = ypool.tile([S, g_cols], F32, tag="ys")
                    nc.scalar.activation(ys[:], zs[g][:], Sigmoid,
                                         bias=bias_sb[0:S, 9:10])
                    nc.sync.dma_start(yr[:, g * g_cols:(g + 1) * g_cols],
                                      ys[:])

    nc.compile()
    return nc


# ---------------------------------------------------------------------------
# Tile-position variant: 32 streams in 16 pairs, each layer = one "wave" of
# 16 CONCURRENT 32x32 tile matmuls (measured ~3.4x the full-array fp32 rate,
# since a 32x32 tile holding blockdiag(W.T x2) does 50% useful MACs vs the
# 128x128 block-diagonal's 12.5%).  Pair p lives at coordinates
# (strip s, free-block fb) of the current [128, 4F] activation tile; the MM
# for a pair is tile_position (32*s, 32*s') with s' = (s+fb) % 4, writing
# PSUM (strip s', free-block s) -- so positions evolve by the invertible map
# (s, fb) -> ((s+fb)%4, s) and all 16 (row, col) tiles are used exactly once
# per wave.  PE drops to ~1 us per 8192-point wave; the scalar engine's
# tanh throughput becomes the bottleneck.
# ---------------------------------------------------------------------------

N_STREAMS = 32
N_PAIRS = 16
L32 = N_CORE // N_STREAMS    # 8192 points per stream


def _pair_positions():
    """pos[layer][p] = (strip, freeblock) for layers 1..10 (post-L0..output)."""
    pos = [{p: (p // 4, p % 4) for p in range(N_PAIRS)}]
    for _ in range(N_HIDDEN + 1):
        nxt = {}
        for p, (s, fb) in pos[-1].items():
            nxt[p] = ((s + fb) % 4, s)
        pos.append(nxt)
    return pos


def format_inputs_tiles(W0, b0, Wh, bh, Wo, bo):
    W0 = np.asarray(W0, np.float32)
    b0 = np.asarray(b0, np.float32)
    Wh = np.asarray(Wh, np.float32)
    bh = np.asarray(bh, np.float32)
    Wo = np.asarray(Wo, np.float32)
    bo = np.asarray(bo, np.float32)

    # L0 stationary [128, 32]: strip c rows 3*sl+f, cols 16*sl+m = W0[m, f]
    w0t32 = np.zeros((128, 32), np.float32)
    for c in range(4):
        for sl in range(2):
            for f in range(3):
                w0t32[32 * c + 3 * sl + f, 16 * sl:16 * sl + W] = W0[:, f]

    # hidden stationaries [128, 32*8]: strip c = blockdiag(Wh[l].T x2)
    wh32 = np.zeros((128, 32 * N_HIDDEN), np.float32)
    for l in range(N_HIDDEN):
        for c in range(4):
            for sl in range(2):
                r = 32 * c + 16 * sl
                wh32[r:r + W, 32 * l + 16 * sl:32 * l + 16 * sl + W] = Wh[l].T

    # output stationary [128, 32]: strip c rows 16*sl+fi, col sl = Wo[0, fi];
    # cols 2..31 zero so the MM writes its full 32-row strip (cost is
    # per-column, so the padding is free and keeps PSUM fully initialized)
    wo32 = np.zeros((128, 32), np.float32)
    for c in range(4):
        for sl in range(2):
            wo32[32 * c + 16 * sl:32 * c + 16 * sl + W, sl] = Wo[0, :]

    # bias table [128, 10]: tanh cols use rows 32c+16sl+m; sigmoid col 9
    # uses rows 32c+sl
    bias32 = np.zeros((128, 10), np.float32)
    for c in range(4):
        for sl in range(2):
            r = 32 * c + 16 * sl
            bias32[r:r + W, 0] = b0
            for l in range(N_HIDDEN):
                bias32[r:r + W, 1 + l] = bh[l]
    bias32[:, 9] = bo[0]
    for c in range(4):
        for sl in range(2):
            pass

    # dense output stationary [128, 8] for tiles2: rhs strip s, slot sl
    # -> out row 2s+sl (one full-array MM replaces 16 tile MMs)
    wod = np.zeros((128, 8), np.float32)
    for s in range(4):
        for sl in range(2):
            wod[32 * s + 16 * sl:32 * s + 16 * sl + W, 2 * s + sl] = Wo[0, :]
    return {"w0t32": w0t32, "wh32": wh32, "wo32": wo32, "bias32": bias32,
            "wod": wod}


def host_pack_x_tiles(x_core, f_cols=512):
    """[n_core, 3] -> packed [24, l32*4]: per wave w, strip b rows 6b+3sl+f,
    free w*4F + a*F + n = x[(2*(4a+b)+sl)*l32 + w*F + n, f]."""
    l32 = x_core.shape[0] // N_STREAMS
    nw = l32 // f_cols
    # [a, b, sl, w, n, f] -> [b, sl, f, w, a, n]
    xv = np.asarray(x_core, np.float32).reshape(4, 4, 2, nw, f_cols, 3)
    out = xv.transpose(1, 2, 5, 3, 0, 4)
    return np.ascontiguousarray(out).reshape(24, nw * 4 * f_cols)


def host_unpack_y_tiles(y_raw, f_cols=512):
    """y_raw [128, l32*4] -> y [n_core, 1] using final pair positions."""
    l32 = y_raw.shape[1] // 4
    nw = l32 // f_cols
    pos_out = _pair_positions()[N_HIDDEN + 1]
    yv = y_raw.reshape(8, nw, 4, f_cols)
    rows = np.array([2 * pos_out[p][0] + sl
                     for p in range(N_PAIRS) for sl in range(2)])
    fbs = np.array([pos_out[p][1] for p in range(N_PAIRS)]).repeat(2)
    y = yv[rows, :, fbs, :]          # [32, nw, f_cols]
    return np.ascontiguousarray(y).reshape(N_STREAMS * l32, 1)


def host_unpack_y_tiles2(y_raw, f_cols=512):
    """dense-Lout y_raw [8*nw, 4F]: row 8w+2s+sl, col fb*F+n'."""
    nw = y_raw.shape[0] // 8
    pos8 = _pair_positions()[N_HIDDEN]
    yv = y_raw.reshape(nw, 4, 2, 4, f_cols)   # [w, s, sl, fb, n']
    l32 = nw * f_cols
    y = np.empty((N_STREAMS, nw, f_cols), np.float32)
    for p in range(N_PAIRS):
        s, fb = pos8[p]
        for sl in range(2):
            y[2 * p + sl] = yv[:, s, sl, fb, :]
    return np.ascontiguousarray(y).reshape(N_STREAMS * l32, 1)


def build_program_tiles(n_core=N_CORE, f_cols=512, repeat=1,
                        num_devices=N_CORES, depth=2, xbufs=None, hbufs=None,
                        ybufs=None, dense_lout=False):
    """Tile-position wave kernel.  One wave = 16 concurrent 32x32 MMs
    (f_cols columns each) into a [128, 4*f_cols] PSUM tile (2 banks at
    f_cols=256), one ACT pass per wave-layer.  `depth` wave-chains are
    interleaved so ACT stays busy while the other chain's MMs run."""
    l32 = n_core // N_STREAMS
    nw = l32 // f_cols
    F = f_cols
    pos = _pair_positions()

    nc = bacc.Bacc("TRN2", target_bir_lowering=False, debug=False,
                   num_devices=num_devices)
    xt_ap = nc.dram_tensor("xt", [24, l32 * 4], F32, kind="ExternalInput").ap()
    w0_ap = nc.dram_tensor("w0t32", [128, 32], F32, kind="ExternalInput").ap()
    wh_ap = nc.dram_tensor("wh32", [128, 32 * N_HIDDEN], F32,
                           kind="ExternalInput").ap()
    wo_ap = nc.dram_tensor("wo32", [128, 32], F32, kind="ExternalInput").ap()
    wod_ap = nc.dram_tensor("wod", [128, 8], F32, kind="ExternalInput").ap()
    bias_ap = nc.dram_tensor("bias32", [128, 10], F32,
                             kind="ExternalInput").ap()
    # only rows 32c+sl of the sigmoid output carry data; ship them packed
    if dense_lout:
        y_ap = nc.dram_tensor("y_raw", [8 * nw, 4 * f_cols], F32,
                              kind="ExternalOutput").ap()
    else:
        y_ap = nc.dram_tensor("y_raw", [8, l32 * 4], F32,
                              kind="ExternalOutput").ap()

    Tanh = mybir.ActivationFunctionType.Tanh
    Sigmoid = mybir.ActivationFunctionType.Sigmoid

    with tile.TileContext(nc) as tc, ExitStack() as ctx:
        consts = ctx.enter_context(tc.tile_pool(name="consts", bufs=1))
        xpool = ctx.enter_context(tc.tile_pool(
            name="xpool", bufs=xbufs or depth + 1))
        hpool = ctx.enter_context(tc.tile_pool(
            name="hpool", bufs=hbufs or 2 * depth + 1))
        ypool = ctx.enter_context(tc.tile_pool(
            name="ypool", bufs=ybufs or depth + 1))
        # one PSUM slot is [128, 4F] with free-block i = one full bank at
        # F=512, so concurrent tile drains never share a (bank, partition)
        zbufs = max(2, 8 // max(1, (4 * f_cols * 4) // 2048))
        zpool = ctx.enter_context(tc.tile_pool(name="zpool", bufs=zbufs,
                                               space="PSUM"))

        w0_sb = consts.tile([128, 32], F32)
        nc.sync.dma_start(w0_sb[:], w0_ap[:])
        wh_sb = consts.tile([128, 32 * N_HIDDEN], F32)
        nc.sync.dma_start(wh_sb[:], wh_ap[:])
        wo_sb = consts.tile([128, 32], F32)
        nc.sync.dma_start(wo_sb[:], wo_ap[:])
        wod_sb = consts.tile([128, 8], F32)
        nc.sync.dma_start(wod_sb[:], wod_ap[:])
        bias_sb = consts.tile([128, 10], F32)
        nc.sync.dma_start(bias_sb[:], bias_ap[:])

        for _rep in range(repeat):
            if dense_lout:
                ystage = ypool.tile([8 * nw, 4 * f_cols], F32, tag="ystage",
                                    bufs=1)
            for w0i in range(0, nw, depth):
                ws = range(w0i, min(w0i + depth, nw))
                xss, hs, zs = {}, {}, {}
                for wv in ws:
                    xs = xpool.tile([128, 4 * F], F32, tag="xs")
                    for b in range(4):
                        nc.sync.dma_start(
                            xs[32 * b:32 * b + 6, :],
                            xt_ap[6 * b:6 * b + 6,
                                  wv * 4 * F:(wv + 1) * 4 * F])
                    xss[wv] = xs

                # ---- layer 0 wave: pair (a,b): tile (b, a), rhs strip b
                # fb a, out (strip a, fb b)
                for wv in ws:
                    z = zpool.tile([128, 4 * F], F32, tag="z")
                    zs[wv] = z
                    for p in range(N_PAIRS):
                        a, b = p // 4, p % 4
                        nc.tensor.matmul(
                            z[32 * a:32 * (a + 1), b * F:(b + 1) * F],
                            lhsT=w0_sb[32 * b:32 * b + 6, :],
                            rhs=xss[wv][32 * b:32 * b + 6,
                                        a * F:(a + 1) * F],
                            start=True, stop=True,
                            tile_position=(32 * b, 32 * a),
                            skip_group_check=True,
                        )
                for wv in ws:
                    h = hpool.tile([128, 4 * F], F32, tag="h")
                    nc.scalar.activation(h[:], zs[wv][:], Tanh,
                                         bias=bias_sb[:, 0:1])
                    hs[wv] = h

                # ---- hidden waves
                for l in range(N_HIDDEN):
                    cur, nxt = pos[l], pos[l + 1]
                    for wv in ws:
                        z = zpool.tile([128, 4 * F], F32, tag="z")
                        zs[wv] = z
                        for p in range(N_PAIRS):
                            s, fb = cur[p]
                            s2 = nxt[p][0]
                            nc.tensor.matmul(
                                z[32 * s2:32 * (s2 + 1), s * F:(s + 1) * F],
                                lhsT=wh_sb[32 * s:32 * (s + 1),
                                           32 * l:32 * (l + 1)],
                                rhs=hs[wv][32 * s:32 * (s + 1),
                                           fb * F:(fb + 1) * F],
                                start=True, stop=True,
                                tile_position=(32 * s, 32 * s2),
                                skip_group_check=True,
                            )
                    for wv in ws:
                        h2 = hpool.tile([128, 4 * F], F32, tag="h")
                        nc.scalar.activation(h2[:], zs[wv][:], Tanh,
                                             bias=bias_sb[:, l + 1:l + 2])
                        hs[wv] = h2

                # ---- output wave
                cur, nxt = pos[N_HIDDEN], pos[N_HIDDEN + 1]
                if dense_lout:
                    # one full-array MM per f_cols chunk -> dense [8, 4F]
                    for wv in ws:
                        zo = zpool.tile([128, 4 * F], F32, tag="z")
                        zs[wv] = zo
                        for c in range(4):
                            nc.tensor.matmul(
                                zo[0:8, c * F:(c + 1) * F],
                                lhsT=wod_sb[:],
                                rhs=hs[wv][:, c * F:(c + 1) * F],
                                start=True, stop=True,
                            )
                    for wv in ws:
                        tmp = ypool.tile([8, 4 * F], F32, tag="ys")
                        nc.vector.tensor_copy(tmp[:], zs[wv][0:8, :])
                        nc.sync.dma_start(
                            ystage[8 * wv:8 * wv + 8, :], tmp[:])
                else:
                    for wv in ws:
                        zo = zpool.tile([128, 4 * F], F32, tag="z")
                        zs[wv] = zo
                        for p in range(N_PAIRS):
                            s, fb = cur[p]
                            s2 = nxt[p][0]
                            nc.tensor.matmul(
                                zo[32 * s2:32 * (s2 + 1), s * F:(s + 1) * F],
                                lhsT=wo_sb[32 * s:32 * (s + 1), :],
                                rhs=hs[wv][32 * s:32 * (s + 1),
                                           fb * F:(fb + 1) * F],
                                start=True, stop=True,
                                tile_position=(32 * s, 32 * s2),
                                skip_group_check=True,
                            )
                    for wv in ws:
                        ys = ypool.tile([128, 4 * F], F32, tag="ys")
                        nc.scalar.activation(ys[:], zs[wv][:], Sigmoid,
                                             bias=bias_sb[:, 9:10])
                        for c in range(4):
                            nc.sync.dma_start(
                                y_ap[2 * c:2 * c + 2,
                                     wv * 4 * F:(wv + 1) * 4 * F],
                                ys[32 * c:32 * c + 2, :])

            if dense_lout:
                ysig = ypool.tile([8 * nw, 4 * f_cols], F32, tag="ysig",
                                  bufs=1)
                nc.scalar.activation(ysig[:], ystage[:], Sigmoid,
                                     bias=bias_sb[0:8 * nw, 9:10])
                nc.sync.dma_start(y_ap[:], ysig[:])

    nc.compile()
    return nc


# ---------------------------------------------------------------------------
# v2 "tiles2": tiles kernel with column-dense output + DMA queue separation.
# Measured 264.5 us/core (vs 396.4 us for build_program_tiles); the scalar
# engine is the roofline: 144 tanh x 1.77 us + 8 sigmoid x 0.49 us = 259 us
# busy (ACT = 1 elem/cycle/lane @ 1.2 GHz, microbench.py).
#
# Changes vs build_program_tiles:
#  1. All DMAs shared the in-order SP queue, so each pair's x prefetch queued
#     behind sigmoid-gated y stores => ~13 us ACT bubble per pair (the whole
#     396-283 gap).  Now x strips alternate SP/GPSIMD queues; y stores ride
#     GPSIMD behind that pair's x.
#  2. Output layer: 8 accumulating float32r full-array matmuls per pair with
#     block-shifted stationaries (wod64) steer chunk c of chain ch to rows
#     16c+8ch+2s+sl of ONE [64, 512] PSUM tile, so sigmoid runs once per
#     pair on free-size 512 (0.49 us) instead of once per wave on 2048
#     (16 x 1.77 us).  f32r (1 cyc/row vs fp32's 4) keeps the output MMs off
#     the pair-boundary critical path; its 2^-13 noise is fine on the final
#     layer (no tanh-chain amplification).  The last hidden tanh writes an
#     F32R-typed tile because the BIR verifier requires f32r MM operands to
#     be pre-rounded to the bf16+bf16 grid.
#  3. Pair p's L0 matmuls are emitted BEFORE pair p-1's output matmuls so
#     the PE always has the next z ready when ACT frees a PSUM slot.
# ---------------------------------------------------------------------------


def format_inputs_tiles2(W0, b0, Wh, bh, Wo, bo):
    base = format_inputs_tiles(W0, b0, Wh, bh, Wo, bo)
    Wo = np.asarray(Wo, np.float32)
    # Pair-dense output stationaries [128, 16]: chain A -> out rows 0..7,
    # chain B -> rows 8..15; row index r = 2s+sl contracts h rows 32s+16sl.
    wodA = np.zeros((128, 16), np.float32)
    wodB = np.zeros((128, 16), np.float32)
    for s in range(4):
        for sl in range(2):
            r0 = 32 * s + 16 * sl
            wodA[r0:r0 + W, 2 * s + sl] = Wo[0, :]
            wodB[r0:r0 + W, 8 + 2 * s + sl] = Wo[0, :]
    base["wodA"] = round_f32r(wodA)
    base["wodB"] = round_f32r(wodB)
    # Column-dense variant: one [128, 64] stationary per (chunk, chain),
    # packed into [128, 512].  Block i = 2c+ch steers chunk c of chain ch to
    # out rows 16c+8ch+2s+sl, col n (all 8 MMs accumulate into [64, 512]).
    wod64 = np.zeros((128, 8 * 64), np.float32)
    for c in range(4):
        for ch in range(2):
            i = 2 * c + ch
            for s in range(4):
                for sl in range(2):
                    r0 = 32 * s + 16 * sl
                    m = 16 * c + 8 * ch + 2 * s + sl
                    wod64[r0:r0 + W, 64 * i + m] = Wo[0, :]
    base["wod64"] = round_f32r(wod64)
    return base


def host_unpack_y_pairsig(y_raw, f_cols=512):
    """y_raw [128, 4F]: row 16p + 8*ch + 2s+sl, col fb*F+n holds the output
    of pair pp (pos8[pp] = (s, fb)), slot sl, wave w = 2p+ch, point n."""
    F = f_cols
    npair = y_raw.shape[0] // 16
    pos8 = _pair_positions()[N_HIDDEN]
    yv = y_raw.reshape(npair, 2, 4, 2, 4, F)    # [p, ch, s, sl, fb, n]
    nwav = 2 * npair
    l32 = nwav * F
    y = np.empty((N_STREAMS, nwav, F), np.float32)
    for pp in range(N_PAIRS):
        s, fb = pos8[pp]
        for sl in range(2):
            y[2 * pp + sl] = yv[:, :, s, sl, fb, :].reshape(nwav, F)
    return np.ascontiguousarray(y).reshape(N_STREAMS * l32, 1)


def host_unpack_y_pairsig64(y_raw, f_cols=512):
    """Column-dense y_raw [64*npair, F]: row 64p + 16fb + 8ch + 2s+sl,
    col n holds pair pp (pos8[pp] = (s, fb)), slot sl, wave 2p+ch."""
    F = f_cols
    npair = y_raw.shape[0] // 64
    pos8 = _pair_positions()[N_HIDDEN]
    yv = y_raw.reshape(npair, 4, 2, 4, 2, F)    # [p, fb, ch, s, sl, n]
    nwav = 2 * npair
    l32 = nwav * F
    y = np.empty((N_STREAMS, nwav, F), np.float32)
    for pp in range(N_PAIRS):
        s, fb = pos8[pp]
        for sl in range(2):
            y[2 * pp + sl] = yv[:, fb, :, s, sl, :].reshape(nwav, F)
    return np.ascontiguousarray(y).reshape(N_STREAMS * l32, 1)


def build_program_tiles2(n_core=N_CORE, f_cols=512, repeat=1,
                         num_devices=N_CORES, xbufs=5, hbufs=7, ybufs=3,
                         out_f32r=True):
    l32 = n_core // N_STREAMS
    nw = l32 // f_cols
    F = f_cols
    assert nw % 2 == 0
    pos = _pair_positions()

    npair = nw // 2
    nc = bacc.Bacc("TRN2", target_bir_lowering=False, debug=False,
                   num_devices=num_devices)
    xt_ap = nc.dram_tensor("xt", [24, l32 * 4], F32, kind="ExternalInput").ap()
    w0_ap = nc.dram_tensor("w0t32", [128, 32], F32, kind="ExternalInput").ap()
    wh_ap = nc.dram_tensor("wh32", [128, 32 * N_HIDDEN], F32,
                           kind="ExternalInput").ap()
    wod64_ap = nc.dram_tensor("wod64", [128, 8 * 64], F32R,
                              kind="ExternalInput").ap()
    bias_ap = nc.dram_tensor("bias32", [128, 10], F32,
                             kind="ExternalInput").ap()
    y_ap = nc.dram_tensor("y_raw", [64 * npair, F], F32,
                          kind="ExternalOutput").ap()

    Tanh = mybir.ActivationFunctionType.Tanh
    Sigmoid = mybir.ActivationFunctionType.Sigmoid
    xq = [nc.sync, nc.gpsimd]       # x strips alternate the two DGE queues
    yq = nc.gpsimd                  # y stores: SWDGE, behind that pair's x

    with tile.TileContext(nc) as tc, ExitStack() as ctx:
        consts = ctx.enter_context(tc.tile_pool(name="consts", bufs=1))
        xpool = ctx.enter_context(tc.tile_pool(name="xpool", bufs=xbufs))
        hpool = ctx.enter_context(tc.tile_pool(name="hpool", bufs=hbufs))
        ypool = ctx.enter_context(tc.tile_pool(name="ypool", bufs=ybufs))
        zpool = ctx.enter_context(tc.tile_pool(name="zpool", bufs=2,
                                               space="PSUM"))

        w0_sb = consts.tile([128, 32], F32)
        nc.sync.dma_start(w0_sb[:], w0_ap[:])
        wh_sb = consts.tile([128, 32 * N_HIDDEN], F32)
        nc.sync.dma_start(wh_sb[:], wh_ap[:])
        wod64_sb = consts.tile([128, 8 * 64], F32R)
        nc.sync.dma_start(wod64_sb[:], wod64_ap[:])
        bias_sb = consts.tile([128, 10], F32)
        nc.sync.dma_start(bias_sb[:], bias_ap[:])

        def load_x(wv):
            xs = xpool.tile([128, 4 * F], F32, tag="xs")
            for b in range(4):
                xq[b % 2].dma_start(
                    xs[32 * b:32 * b + 6, :],
                    xt_ap[6 * b:6 * b + 6, wv * 4 * F:(wv + 1) * 4 * F])
            return xs

        def l0_wave(xs):
            z = zpool.tile([128, 4 * F], F32, tag="z")
            for p in range(N_PAIRS):
                a, b = p // 4, p % 4
                nc.tensor.matmul(
                    z[32 * a:32 * (a + 1), b * F:(b + 1) * F],
                    lhsT=w0_sb[32 * b:32 * b + 6, :],
                    rhs=xs[32 * b:32 * b + 6, a * F:(a + 1) * F],
                    start=True, stop=True,
                    tile_position=(32 * b, 32 * a),
                    skip_group_check=True,
                )
            return z

        def hidden_wave(h, l):
            cur, nxt = pos[l], pos[l + 1]
            z = zpool.tile([128, 4 * F], F32, tag="z")
            for p in range(N_PAIRS):
                s, fb = cur[p]
                s2 = nxt[p][0]
                nc.tensor.matmul(
                    z[32 * s2:32 * (s2 + 1), s * F:(s + 1) * F],
                    lhsT=wh_sb[32 * s:32 * (s + 1), 32 * l:32 * (l + 1)],
                    rhs=h[32 * s:32 * (s + 1), fb * F:(fb + 1) * F],
                    start=True, stop=True,
                    tile_position=(32 * s, 32 * s2),
                    skip_group_check=True,
                )
            return z

        def act(z, col, tag="h", dtype=F32):
            h = hpool.tile([128, 4 * F], dtype, tag=tag)
            nc.scalar.activation(h[:], z[:], Tanh,
                                 bias=bias_sb[:, col:col + 1])
            return h

        def out_mms(hA, hB):
            """Column-dense output: 8 accumulating f32r MMs steer chunk c of
            chain ch to rows 16c+8ch+..., giving [64, F] — so the sigmoid's
            free size is F, not 4F.  hA/hB are F32R (rounded on ACT write),
            so the MMs run at 1 cyc/row and stay off the critical path."""
            zo = zpool.tile([64, F], F32, tag="z")
            for c in range(4):
                for ch, h in enumerate((hA, hB)):
                    i = 2 * c + ch
                    nc.tensor.matmul(
                        zo[0:64, :],
                        lhsT=wod64_sb[:, 64 * i:64 * (i + 1)],
                        rhs=h[:, c * F:(c + 1) * F],
                        start=(i == 0), stop=(i == 7),
                    )
            return zo

        def sig_out(zo, p, last=False):
            ys = ypool.tile([64, F], F32, tag="ys")
            nc.scalar.activation(ys[:], zo[:], Sigmoid,
                                 bias=bias_sb[0:64, 9:10])
            if last:    # split the tail store across both queues
                yq.dma_start(y_ap[64 * p:64 * p + 32, :], ys[0:32, :])
                nc.sync.dma_start(y_ap[64 * p + 32:64 * p + 64, :],
                                  ys[32:64, :])
            else:
                yq.dma_start(y_ap[64 * p:64 * p + 64, :], ys[:])

        for _rep in range(repeat):
            prev = None     # (zo emitted?, hA8, hB8, pair_idx)
            for p in range(nw // 2):
                wA, wB = 2 * p, 2 * p + 1
                xsA = load_x(wA)
                xsB = load_x(wB)
                zA = l0_wave(xsA)
                if prev is not None:
                    zo = out_mms(prev[0], prev[1])
                hA = act(zA, 0)
                if prev is not None:
                    sig_out(zo, prev[2])
                zB = l0_wave(xsB)
                hB = act(zB, 0)
                for l in range(N_HIDDEN):
                    # the last hidden tanh feeds only the f32r output MMs;
                    # write it pre-rounded to the f32r grid
                    dt = F32R if (out_f32r and l == N_HIDDEN - 1) else F32
                    zA = hidden_wave(hA, l)
                    hA = act(zA, l + 1, dtype=dt)
                    zB = hidden_wave(hB, l)
                    hB = act(zB, l + 1, dtype=dt)
                prev = (hA, hB, p)
            zo = out_mms(prev[0], prev[1])
            sig_out(zo, prev[2], last=True)

    nc.compile()
    return nc


_RUNNER_CACHE = {}
L0_MODE = "host_t"
KERNEL_MODE = "tiles2"  # "tiles2" | "tiles" | "block"


def host_transpose_x(x_core):
    """[n_core, 3] -> [24, n_core/S]: partition 8f+j = feature f of stream j."""
    L = x_core.shape[0] // S
    return np.ascontiguousarray(
        x_core.reshape(S, L, 3).transpose(2, 0, 1).reshape(24, L))


def make_in_maps(x, consts, l0_mode=None):
    """Per-core input maps from the full x [N_FULL, 3] + formatted weights."""
    l0_mode = l0_mode or L0_MODE
    in_maps = []
    for c in range(N_CORES):
        xc = x[c * N_CORE:(c + 1) * N_CORE]
        if l0_mode == "host_t":
            m = {"xt": host_transpose_x(xc)}
        else:
            m = {"x": np.ascontiguousarray(xc)}
        m.update(consts)
        in_maps.append(m)
    return in_maps


def make_runner(nc, n_cores=N_CORES):
    """Build a reusable jitted PJRT runner for the SPMD program (mirrors
    bass2jax.run_bass_via_pjrt's multi-core path, minus output donation, so
    the NEFF compile is paid once and later calls are just execution)."""
    import jax
    from jax.sharding import Mesh, PartitionSpec, NamedSharding
    from jax.experimental.shard_map import shard_map
    from concourse import bass2jax

    bass2jax.install_neuronx_cc_hook()
    partition_name = (nc.partition_id_tensor.name
                      if nc.partition_id_tensor else None)
    in_names, out_names, out_avals = [], [], []
    for alloc in nc.m.functions[0].allocations:
        if not isinstance(alloc, mybir.MemoryLocationSet):
            continue
        name = alloc.memorylocations[0].name
        if alloc.kind == "ExternalInput":
            if name != partition_name:
                in_names.append(name)
        elif alloc.kind == "ExternalOutput":
            out_names.append(name)
            out_avals.append(jax.core.ShapedArray(
                tuple(alloc.tensor_shape), mybir.dt.np(alloc.dtype)))
    n_params = len(in_names)
    all_in_names = list(in_names) + list(out_names)
    if partition_name is not None:
        all_in_names.append(partition_name)

    def _body(*args):
        operands = list(args)
        if partition_name is not None:
            operands.append(bass2jax.partition_id_tensor())
        outs = bass2jax._bass_exec_p.bind(
            *operands,
            out_avals=tuple(out_avals),
            in_names=tuple(all_in_names),
            out_names=tuple(out_names),
            lowering_input_output_aliases=(),
            sim_require_finite=True,
            sim_require_nnan=True,
            nc=nc,
        )
        return tuple(outs)

    devices = jax.devices()[:n_cores]
    mesh = Mesh(np.asarray(devices), ("core",))
    n_outs = len(out_names)
    in_specs = (PartitionSpec("core"),) * (n_params + n_outs)
    out_specs = (PartitionSpec("core"),) * n_outs
    fn = jax.jit(shard_map(_body, mesh=mesh, in_specs=in_specs,
                           out_specs=out_specs, check_rep=False),
                 keep_unused=True)
    sharding = NamedSharding(mesh, PartitionSpec("core"))

    def prepare(in_maps):
        concat_in = [
            np.concatenate([np.asarray(in_maps[c][n])
                            for c in range(n_cores)], axis=0)
            for n in in_names
        ]
        concat_zero = [
            np.zeros((n_cores * a.shape[0],) + tuple(a.shape[1:]), a.dtype)
            for a in out_avals
        ]
        return [jax.device_put(a, sharding) for a in concat_in + concat_zero]

    return fn, prepare, out_names


def _get_runner(key=(N_CORE, 1024, 512, 1, L0_MODE)):
    if key not in _RUNNER_CACHE:
        n_core, g_cols, f_cols, repeat, l0_mode = key
        nc = build_program(n_core, g_cols, f_cols, repeat, l0_mode=l0_mode)
        _RUNNER_CACHE[key] = make_runner(nc)
    return _RUNNER_CACHE[key]


def _get_runner_tiles(key=(N_CORE, 512, 1)):
    if key not in _RUNNER_CACHE:
        n_core, f_cols, repeat = key
        nc = build_program_tiles(n_core, f_cols, repeat)
        _RUNNER_CACHE[key] = make_runner(nc)
    return _RUNNER_CACHE[key]


def _get_runner_tiles2(key=(N_CORE, 512, 1)):
    if ("t2",) + key not in _RUNNER_CACHE:
        n_core, f_cols, repeat = key
        nc = build_program_tiles2(n_core, f_cols, repeat)
        _RUNNER_CACHE[("t2",) + key] = make_runner(nc)
    return _RUNNER_CACHE[("t2",) + key]


def kernel(x, W0, b0, Wh, bh, Wo, bo):
    import jax
    x = np.asarray(x, np.float32)
    assert x.shape == (N_FULL, 3), x.shape
    if KERNEL_MODE == "tiles2":
        consts = format_inputs_tiles2(W0, b0, Wh, bh, Wo, bo)
        fn, prepare, out_names = _get_runner_tiles2()
        in_maps = []
        for c in range(N_CORES):
            m = {"xt": host_pack_x_tiles(x[c * N_CORE:(c + 1) * N_CORE])}
            m.update({k: consts[k] for k in
                      ("w0t32", "wh32", "wod64", "bias32")})
            in_maps.append(m)
        args = prepare(in_maps)
        outs = fn(*args)
        jax.block_until_ready(outs)
        y_raw = np.asarray(outs[out_names.index("y_raw")])
        y_raw = y_raw.reshape(N_CORES, 512, -1)
        y = np.concatenate(
            [host_unpack_y_pairsig64(y_raw[c]) for c in range(N_CORES)],
            axis=0)
        return np.ascontiguousarray(y.astype(np.float32))
    if KERNEL_MODE == "tiles":
        consts = format_inputs_tiles(W0, b0, Wh, bh, Wo, bo)
        fn, prepare, out_names = _get_runner_tiles()
        in_maps = []
        for c in range(N_CORES):
            m = {"xt": host_pack_x_tiles(x[c * N_CORE:(c + 1) * N_CORE])}
            m.update(consts)
            in_maps.append(m)
        args = prepare(in_maps)
        outs = fn(*args)
        jax.block_until_ready(outs)
        y_raw = np.asarray(outs[out_names.index("y_raw")])
        y_raw = y_raw.reshape(N_CORES, 8, L32 * 4)
        y = np.concatenate(
            [host_unpack_y_tiles(y_raw[c]) for c in range(N_CORES)], axis=0)
        return np.ascontiguousarray(y.astype(np.float32))
    consts = format_inputs(W0, b0, Wh, bh, Wo, bo)
    fn, prepare, out_names = _get_runner()
    args = prepare(make_in_maps(x, consts))
    outs = fn(*args)
    jax.block_until_ready(outs)
    y = np.asarray(outs[out_names.index("y")])
    return np.ascontiguousarray(y.reshape(N_FULL, 1).astype(np.float32))

